# revision 11
# baseline (speedup 1.0000x reference)
"""Trainium2 Bass kernel for sliding-window causal attention block.

Reference computation (B=2, S=2048, D=1024, H=16, hd=64, WINDOW=256):
    c = x @ w_attn + b_attn ; q,k,v = split(c)
    present = stack([k, v]) as [B,2,H,S,hd]
    att = softmax(mask(q k^T / sqrt(hd))) @ v
    out = att @ w_proj + b_proj

Sharding: 8 cores = 2 batches x 4 head-groups (4 heads each).
Per core: QKV projection for its 256 q/k/v features (column-sharded
w_attn), attention for its 4 heads, and a partial out-projection
(row-sharded w_proj). Host sums the 4 partials per batch and adds
b_proj exactly.

Layout strategy on-core:
  x[b] is PE-transposed once to xT [D, S] (f32r, 4 transposes batched
  per PSUM bank, eviction split across DVE/ACT).
  Q^T, K^T produced feature-major [feat, S] (f32r), V natural [S, feat].
  Scores are computed directly transposed: S^T[kpos, q] tile per
  128-kpos chunk covering its 384 valid q columns (window 256 spans 3
  q-tiles), so softmax needs no P transposes. Masks: causal triangle on
  the left third, all-valid middle, anti-causal triangle right.
  exp on ACT -> P^T fp16; AV matmul uses V|ones fp16 where columns
  64:128 are all ones, so the softmax denominator lands replicated on
  PSUM partitions 64:128 -> 64-lane reciprocal + multiply on DVE.
  O^T f32r; out-projection back to natural [S, D] layout.
  DMAs are batched aggressively (the SP sequencer pays ~0.6us per DMA
  instruction): 8 input DMAs for x, 1 for all of wq/wk/wv, single
  per-head DMAs for the present k/v outputs, 2-S-tile DMAs for out.
"""

import sys

sys.path.insert(0, "/opt/trn_rl_repo")

import numpy as np

import concourse.bass as bass  # noqa: F401  (bass must import before bacc)
import concourse.mybir as mybir
from concourse import bacc
from concourse.tile import TileContext
from concourse.bass_utils import run_bass_kernel_spmd

F32 = mybir.dt.float32
F32R = mybir.dt.float32r
F16 = mybir.dt.float16

B, S, D = 2, 2048, 1024
N_HEAD = 16
HD = 64
WINDOW = 256
NCORES = 8
HPC = N_HEAD // 4  # heads per core = 4
FPC = HPC * HD  # features per core = 256
NQT = S // 128  # 16 q/kpos tiles
NDC = D // 128  # 8 contraction chunks
SCALE = 1.0 / np.sqrt(HD)

_CACHE = {}


def _build_program():
    nc = bacc.Bacc("TRN2", target_bir_lowering=False, debug=False,
                   num_devices=NCORES)

    # ---- DRAM I/O ----
    xb = nc.dram_tensor("xb", [S, D], F32R, kind="ExternalInput")
    # wq | wk | wv, each pre-chunked to [128, NDC*FPC]
    wqkv = nc.dram_tensor("wqkv", [128, 3 * NDC * FPC], F32R,
                          kind="ExternalInput")
    wp = nc.dram_tensor("wp", [128, 2 * D], F32R, kind="ExternalInput")
    bqk = nc.dram_tensor("bqk", [128, 4], F32, kind="ExternalInput")
    bv = nc.dram_tensor("bv", [1, FPC], F32R, kind="ExternalInput")
    ident_r = nc.dram_tensor("ident_r", [128, 128], F32R, kind="ExternalInput")
    madd = nc.dram_tensor("madd", [128, 384], F32R, kind="ExternalInput")
    onesr = nc.dram_tensor("onesr", [1, 128], F32R, kind="ExternalInput")

    outp = nc.dram_tensor("outp", [S, D], F32, kind="ExternalOutput")
    pk = nc.dram_tensor("pk", [HPC, S, HD], F32, kind="ExternalOutput")
    pv = nc.dram_tensor("pv", [HPC, S, HD], F32, kind="ExternalOutput")

    Exp = mybir.ActivationFunctionType.Exp
    Ident = mybir.ActivationFunctionType.Identity

    with TileContext(nc) as tc:
        with tc.tile_pool(name="const", bufs=1) as cpool, \
             tc.tile_pool(name="qkv", bufs=1) as qkv, \
             tc.tile_pool(name="ot", bufs=1) as otp:

            # identity first (x transposes depend on it)
            idr_sb = cpool.tile([128, 128], F32R, tag="idr")
            nc.sync.dma_start(out=idr_sb[:], in_=ident_r[:])

            # persistent activations
            qt_sb = [qkv.tile([128, S], F32R, tag=f"qt{i}", name=f"qt{i}")
                     for i in range(2)]
            kt_sb = [qkv.tile([128, S], F32R, tag=f"kt{i}", name=f"kt{i}")
                     for i in range(2)]
            v_sb = qkv.tile([128, NQT * FPC], F32, tag="v")
            # V|ones fp16 per head: head hl block [128, NQT*128];
            # cols 64:128 of each chunk stay 1.0 (denominator trick)
            vb_sb = qkv.tile([128, HPC * NQT * 128], F16, tag="vb")
            ot_sb = [otp.tile([128, S], F32R, tag=f"ot{i}", name=f"ot{i}")
                     for i in range(2)]

            nc.gpsimd.memset(vb_sb[:], 1.0)

            # ---- stage 1+2: x load/transpose + QKV projection ----
            with tc.tile_pool(name="xt", bufs=1) as xtp, \
                 tc.tile_pool(name="xload", bufs=5) as xlp, \
                 tc.tile_pool(name="tp_ps", bufs=2, space="PSUM") as tpps, \
                 tc.tile_pool(name="qk_ps", bufs=4, space="PSUM") as qkps, \
                 tc.tile_pool(name="vp_ps", bufs=2, space="PSUM") as vpps:

                xt_sb = xtp.tile([128, NDC * S], F32R, tag="xt")
                xt3 = xt_sb[:].rearrange("p (dc s) -> p dc s", dc=NDC)

                # prefetch the first S-group's x tiles ahead of the bulky
                # weight transfer so PE transposes start immediately
                x_pre = {}
                for st in range(4):
                    x_t = xlp.tile([128, D], F32R, tag="x", name=f"x{st}")
                    nc.sync.dma_start(out=x_t[:],
                                      in_=xb[st * 128:(st + 1) * 128, :])
                    x_pre[st] = x_t

                # weights next in emission (small, DMA queue drains while
                # x tiles stream)
                wqkv_sb = cpool.tile([128, 3 * NDC * FPC], F32R, tag="wqkv")
                nc.sync.dma_start(out=wqkv_sb[:], in_=wqkv[:])
                wq_sb = wqkv_sb[:, 0:NDC * FPC]
                wk_sb = wqkv_sb[:, NDC * FPC:2 * NDC * FPC]
                wv_sb = wqkv_sb[:, 2 * NDC * FPC:3 * NDC * FPC]
                bqk_sb = cpool.tile([128, 4], F32, tag="bqk")
                nc.sync.dma_start(out=bqk_sb[:], in_=bqk[:])
                bv_sb = cpool.tile([1, FPC], F32R, tag="bv")
                nc.sync.dma_start(out=bv_sb[:], in_=bv[:])
                madd_sb = cpool.tile([128, 384], F32R, tag="madd")
                nc.sync.dma_start(out=madd_sb[:], in_=madd[:])
                on_sb = cpool.tile([1, 128], F32R, tag="on")
                nc.sync.dma_start(out=on_sb[:], in_=onesr[:])

                # interleave x load/transpose with QKV so PE never waits
                # for the full x transfer (PE streams are in-order)
                for sc in range(4):
                    for st in range(4 * sc, 4 * sc + 4):
                        if st in x_pre:
                            x_t = x_pre.pop(st)
                        else:
                            x_t = xlp.tile([128, D], F32R, tag="x",
                                           name=f"x{st}")
                            nc.sync.dma_start(
                                out=x_t[:],
                                in_=xb[st * 128:(st + 1) * 128, :])
                        for dg in range(2):
                            ps = tpps.tile([128, 512], F32R, tag="tp",
                                           name=f"tp{st}_{dg}")
                            for j in range(4):
                                dc = dg * 4 + j
                                nc.tensor.transpose(
                                    ps[:, j * 128:(j + 1) * 128],
                                    x_t[:, dc * 128:(dc + 1) * 128],
                                    idr_sb[:])
                            dst = xt3[:, dg * 4:(dg + 1) * 4,
                                      st * 128:st * 128 + 128]
                            srcp = ps[:].rearrange("p (j s) -> p j s", j=4)
                            if (st + dg) % 2 == 0:
                                nc.vector.tensor_copy(dst, srcp)
                            else:
                                nc.scalar.copy(dst, srcp)

                    # Q^T / K^T columns for this 512-wide S group.
                    # Q evicts on ACT (+bias), K evicts on DVE (+bias).
                    for wi, (w_sb, dstt) in enumerate(((wq_sb, qt_sb),
                                                       (wk_sb, kt_sb))):
                        for ft in range(2):
                            psq = qkps.tile([128, 512], F32, tag="qk",
                                            name=f"qkps{sc}_{wi}{ft}")
                            for dc in range(NDC):
                                lhsT = w_sb[:, dc * FPC + ft * 128:
                                            dc * FPC + ft * 128 + 128]
                                nc.tensor.matmul(
                                    psq[:], lhsT,
                                    xt_sb[:, dc * S + sc * 512:
                                          dc * S + sc * 512 + 512],
                                    start=(dc == 0), stop=(dc == NDC - 1))
                            bias_ap = bqk_sb[:, 2 * wi + ft: 2 * wi + ft + 1]
                            dslc = dstt[ft][:, sc * 512:(sc + 1) * 512]
                            if wi == 0:
                                nc.scalar.activation(dslc, psq[:], Ident,
                                                     bias=bias_ap)
                            else:
                                nc.vector.tensor_scalar_add(dslc, psq[:],
                                                            bias_ap)

                    # V natural + fused bias for these 4 S-tiles
                    for st in range(4 * sc, 4 * sc + 4):
                        vp = vpps.tile([128, FPC], F32, tag="vp",
                                       name=f"vp{st}")
                        for dc in range(NDC):
                            nc.tensor.matmul(
                                vp[:],
                                xt_sb[:, dc * S + st * 128:
                                      dc * S + st * 128 + 128],
                                wv_sb[:, dc * FPC:(dc + 1) * FPC],
                                start=(dc == 0), stop=False)
                        nc.tensor.matmul(vp[:], on_sb[:], bv_sb[:],
                                         start=False, stop=True)
                        nc.scalar.copy(v_sb[:, st * FPC:(st + 1) * FPC],
                                       vp[:])
                        srcv = vp[:].rearrange("p (hl c) -> p hl c", hl=HPC)
                        dst3 = vb_sb[:].rearrange(
                            "p (hl t) -> p hl t", hl=HPC)[
                            :, :, st * 128: st * 128 + HD]
                        if st % 2 == 0:
                            nc.vector.tensor_copy(dst3, srcv)
                        else:
                            nc.scalar.copy(dst3, srcv)

            # pools that reuse the space freed by xt: wp, K-natural
            # collector, attention transients, out staging
            with tc.tile_pool(name="late", bufs=1) as late, \
                 tc.tile_pool(name="pt", bufs=4) as ptp, \
                 tc.tile_pool(name="rs", bufs=4) as rsp, \
                 tc.tile_pool(name="osb", bufs=2) as osbp:

                wp_sb = late.tile([128, 2 * D], F32R, tag="wp")
                nc.sync.dma_start(out=wp_sb[:], in_=wp[:])
                kn_sb = late.tile([128, 2 * NQT * 128], F32, tag="kn")

                # ---- stage 3: K natural + present outputs ----
                with tc.tile_pool(name="kn_ps", bufs=3, space="PSUM") as knps:
                    for ft in range(2):
                        for st in range(NQT):
                            kp = knps.tile([128, 128], F32R, tag="knp")
                            nc.tensor.transpose(
                                kp[:], kt_sb[ft][:, st * 128:(st + 1) * 128],
                                idr_sb[:])
                            dstk = kn_sb[:, ft * S + st * 128:
                                         ft * S + st * 128 + 128]
                            if st % 2 == 0:
                                nc.scalar.copy(dstk, kp[:].bitcast(F32))
                            else:
                                nc.vector.tensor_copy(dstk, kp[:].bitcast(F32))
                    # one DMA per head
                    kn4 = kn_sb[:].rearrange(
                        "p (ft st h d) -> p ft st h d", ft=2, st=NQT, h=2)
                    for ft in range(2):
                        for h2 in range(2):
                            nc.sync.dma_start(
                                out=pk[2 * ft + h2, :, :].rearrange(
                                    "(st p) d -> p st d", p=128),
                                in_=kn4[:, ft, :, h2, :])
                    v4 = v_sb[:].rearrange(
                        "p (st hl d) -> p st hl d", st=NQT, hl=HPC)
                    for hl in range(HPC):
                        nc.sync.dma_start(
                            out=pv[hl, :, :].rearrange(
                                "(st p) d -> p st d", p=128),
                            in_=v4[:, :, hl, :])

                # ---- stage 4: attention per head (software-pipelined).
                # The last head also interleaves the out-projection so PE
                # fills exp-latency stalls with useful work. ----
                def attention_head(hl, stps, ops, norm_lag, per_chunk_hook):
                    ft, po = hl // 2, (hl % 2) * 64
                    kth = kt_sb[ft]
                    qth = qt_sb[ft]
                    oth = ot_sb[ft]
                    vbh = vb_sb[:, hl * NQT * 128:(hl + 1) * NQT * 128]

                    pts = [None] * NQT
                    osums = [None] * NQT

                    def qk_exp_mask(c):
                        qw = min(384, S - c * 128)
                        sp = stps.tile([128, 384], F32, tag="sp",
                                       name=f"sp{hl}_{c}")
                        nc.tensor.matmul(
                            sp[:, :qw],
                            kth[po:po + 64, c * 128:(c + 1) * 128],
                            qth[po:po + 64, c * 128:c * 128 + qw],
                            start=True, stop=True)
                        pt = ptp.tile([128, 384], F16, tag="pt",
                                      name=f"pt{hl}_{c}")
                        nc.scalar.activation(pt[:, :qw], sp[:, :qw], Exp)
                        # zero masked entries on GPSIMD (idle engine):
                        # left third: keep where ql - kl >= 0
                        nc.gpsimd.affine_select(
                            out=pt[:, 0:128], in_=pt[:, 0:128],
                            compare_op=mybir.AluOpType.is_ge, fill=0.0,
                            base=0, pattern=[[1, 128]], channel_multiplier=-1)
                        # right third: keep where kl - ql - 1 >= 0
                        if qw > 256:
                            nc.gpsimd.affine_select(
                                out=pt[:, 256:qw], in_=pt[:, 256:qw],
                                compare_op=mybir.AluOpType.is_ge, fill=0.0,
                                base=-1, pattern=[[-1, qw - 256]],
                                channel_multiplier=1)
                        pts[c] = pt

                    def osum(c):
                        if osums[c] is None:
                            osums[c] = ops.tile([128, 128], F32, tag="o",
                                                name=f"o{hl}_{c}")
                        return osums[c]

                    def normalize(c):
                        o_cur = osums[c]
                        rec = rsp.tile([64, 128], F32, tag="rec",
                                       name=f"rec{hl}_{c}")
                        nc.vector.reciprocal(rec[:], o_cur[64:128, :])
                        nc.vector.tensor_mul(
                            oth[po:po + 64, c * 128:(c + 1) * 128],
                            o_cur[0:64, :], rec[:])
                        osums[c] = None

                    qk_exp_mask(0)
                    qk_exp_mask(1)
                    for c in range(NQT):
                        if c + 2 < NQT:
                            qk_exp_mask(c + 2)
                        pt = pts[c]
                        qw = min(384, S - c * 128)
                        lhsT = vbh[:, c * 128:(c + 1) * 128]
                        nc.tensor.matmul(osum(c)[:], lhsT, pt[:, 0:128],
                                         start=(c == 0), stop=True)
                        if c + 1 < NQT:
                            nc.tensor.matmul(osum(c + 1)[:], lhsT,
                                             pt[:, 128:256],
                                             start=(c == 0), stop=False)
                        if c + 2 < NQT and qw > 256:
                            nc.tensor.matmul(osum(c + 2)[:], lhsT,
                                             pt[:, 256:384],
                                             start=True, stop=False)
                        # normalization lags the AV matmuls so the DVE
                        # stream never round-trips against PE
                        if c >= norm_lag:
                            normalize(c - norm_lag)
                            if per_chunk_hook is not None:
                                per_chunk_hook(c - norm_lag)
                        pts[c] = None
                    for c in range(NQT - norm_lag, NQT):
                        normalize(c)
                        if per_chunk_hook is not None:
                            per_chunk_hook(c)

                osb_state = {}

                def outproj_tile(st, opps):
                    # called once per q-tile st (in order) after all heads
                    # normalized it
                    st2, i = st // 2, st % 2
                    if i == 0:
                        osb_state["t"] = osbp.tile([128, 2 * D], F32,
                                                   tag="osb",
                                                   name=f"osb{st2}")
                    o_t = osb_state["t"]
                    for half in range(2):
                        op = opps.tile([128, 512], F32, tag="op",
                                       name=f"op{st}_{half}")
                        nc.tensor.matmul(
                            op[:], ot_sb[0][:, st * 128:(st + 1) * 128],
                            wp_sb[:, half * 512: half * 512 + 512],
                            start=True, stop=False)
                        nc.tensor.matmul(
                            op[:], ot_sb[1][:, st * 128:(st + 1) * 128],
                            wp_sb[:, D + half * 512: D + half * 512 + 512],
                            start=False, stop=True)
                        dsl = o_t[:, i * D + half * 512:
                                  i * D + (half + 1) * 512]
                        if half == 0:
                            nc.scalar.copy(dsl, op[:])
                        else:
                            nc.vector.tensor_copy(dsl, op[:])
                    if i == 1:
                        nc.sync.dma_start(
                            out=outp[st2 * 256:(st2 + 1) * 256, :].rearrange(
                                "(j p) d -> p j d", p=128),
                            in_=o_t[:].rearrange("p (j d) -> p j d", j=2))

                with tc.tile_pool(name="st_ps", bufs=3, space="PSUM") as stps, \
                     tc.tile_pool(name="o_ps", bufs=5, space="PSUM") as ops:
                    for hl in range(HPC - 1):
                        attention_head(hl, stps, ops, 2, None)

                with tc.tile_pool(name="st_ps2", bufs=2, space="PSUM") as stps, \
                     tc.tile_pool(name="o_ps2", bufs=4, space="PSUM") as ops, \
                     tc.tile_pool(name="op_ps", bufs=2, space="PSUM") as opps:
                    attention_head(HPC - 1, stps, ops, 1,
                                   lambda st: outproj_tile(st, opps))

    nc.compile()
    return nc


def _prep_in_maps(x, w_attn, b_attn, w_proj):
    """Per-core input dicts (host-side sharding + layout prep)."""
    x = np.ascontiguousarray(np.asarray(x, dtype=np.float32))
    w_attn = np.asarray(w_attn, dtype=np.float32)
    b_attn = np.asarray(b_attn, dtype=np.float32)
    w_proj = np.asarray(w_proj, dtype=np.float32)

    ident = np.eye(128, dtype=np.float32)
    ql = np.arange(128)[None, :]
    kl = np.arange(128)[:, None]
    neg = np.float32(-1e30)
    madd = np.concatenate(
        [np.where(ql >= kl, np.float32(0), neg),
         np.zeros((128, 128), np.float32),
         np.where(ql < kl, np.float32(0), neg)], axis=1).astype(np.float32)
    onesr = np.ones((1, 128), dtype=np.float32)

    def chunk_w(w_cols):  # [D, FPC] -> [128, NDC*FPC]
        return w_cols.reshape(NDC, 128, FPC).transpose(1, 0, 2).reshape(
            128, NDC * FPC)

    in_maps = []
    for core in range(NCORES):
        b, hg = core // 4, core % 4
        cols = slice(hg * FPC, (hg + 1) * FPC)
        kcols = slice(D + hg * FPC, D + (hg + 1) * FPC)
        vcols = slice(2 * D + hg * FPC, 2 * D + (hg + 1) * FPC)
        rows = slice(hg * FPC, (hg + 1) * FPC)
        wqkv = np.concatenate(
            [chunk_w(w_attn[:, cols] * np.float32(SCALE)),
             chunk_w(w_attn[:, kcols]),
             chunk_w(w_attn[:, vcols])], axis=1)
        bqk = np.stack(
            [(b_attn[cols] * np.float32(SCALE)).reshape(2, 128)[0],
             (b_attn[cols] * np.float32(SCALE)).reshape(2, 128)[1],
             b_attn[kcols].reshape(2, 128)[0],
             b_attn[kcols].reshape(2, 128)[1]], axis=1)
        in_maps.append({
            "xb": x[b],
            "wqkv": np.ascontiguousarray(wqkv),
            "wp": np.ascontiguousarray(
                w_proj[rows, :].reshape(2, 128, D).transpose(1, 0, 2).reshape(
                    128, 2 * D)),
            "bqk": np.ascontiguousarray(bqk),
            "bv": b_attn[vcols].reshape(1, FPC).copy(),
            "ident_r": ident,
            "madd": madd,
            "onesr": onesr,
        })
    return in_maps


def kernel(x, w_attn, b_attn, w_proj, b_proj):
    if "nc" not in _CACHE:
        _CACHE["nc"] = _build_program()
    nc = _CACHE["nc"]

    in_maps = _prep_in_maps(x, w_attn, b_attn, w_proj)
    res = run_bass_kernel_spmd(nc, in_maps, core_ids=list(range(NCORES)))

    b_proj = np.asarray(b_proj, dtype=np.float32)
    out = np.zeros((B, S, D), dtype=np.float32)
    present = np.zeros((B, 2, N_HEAD, S, HD), dtype=np.float32)
    for core in range(NCORES):
        b, hg = core // 4, core % 4
        r = res.results[core]
        out[b] += r["outp"]
        present[b, 0, hg * HPC:(hg + 1) * HPC] = r["pk"]
        present[b, 1, hg * HPC:(hg + 1) * HPC] = r["pv"]
    out += b_proj
    return out, present


# revision 12
# speedup vs baseline: 1.0085x; 1.0085x over previous
"""Trainium2 Bass kernel for sliding-window causal attention block.

Reference computation (B=2, S=2048, D=1024, H=16, hd=64, WINDOW=256):
    c = x @ w_attn + b_attn ; q,k,v = split(c)
    present = stack([k, v]) as [B,2,H,S,hd]
    att = softmax(mask(q k^T / sqrt(hd))) @ v
    out = att @ w_proj + b_proj

Sharding: 8 cores = 2 batches x 4 head-groups (4 heads each).
Per core: QKV projection for its 256 q/k/v features (column-sharded
w_attn), attention for its 4 heads, and a partial out-projection
(row-sharded w_proj). Host sums the 4 partials per batch and adds
b_proj exactly.

Layout strategy on-core:
  x[b] is PE-transposed once to xT [D, S] (f32r, 4 transposes batched
  per PSUM bank, eviction split across DVE/ACT).
  Q^T, K^T produced feature-major [feat, S] (f32r), V natural [S, feat].
  Scores are computed directly transposed: S^T[kpos, q] tile per
  128-kpos chunk covering its 384 valid q columns (window 256 spans 3
  q-tiles), so softmax needs no P transposes. Masks: causal triangle on
  the left third, all-valid middle, anti-causal triangle right.
  exp on ACT -> P^T fp16; AV matmul uses V|ones fp16 where columns
  64:128 are all ones, so the softmax denominator lands replicated on
  PSUM partitions 64:128 -> 64-lane reciprocal + multiply on DVE.
  O^T f32r; out-projection back to natural [S, D] layout.
  DMAs are batched aggressively (the SP sequencer pays ~0.6us per DMA
  instruction): 8 input DMAs for x, 1 for all of wq/wk/wv, single
  per-head DMAs for the present k/v outputs, 2-S-tile DMAs for out.
"""

import sys

sys.path.insert(0, "/opt/trn_rl_repo")

import numpy as np

import concourse.bass as bass  # noqa: F401  (bass must import before bacc)
import concourse.mybir as mybir
from concourse import bacc
from concourse.tile import TileContext
from concourse.bass_utils import run_bass_kernel_spmd

F32 = mybir.dt.float32
F32R = mybir.dt.float32r
F16 = mybir.dt.float16

B, S, D = 2, 2048, 1024
N_HEAD = 16
HD = 64
WINDOW = 256
NCORES = 8
HPC = N_HEAD // 4  # heads per core = 4
FPC = HPC * HD  # features per core = 256
NQT = S // 128  # 16 q/kpos tiles
NDC = D // 128  # 8 contraction chunks
SCALE = 1.0 / np.sqrt(HD)

_CACHE = {}


def _build_program():
    nc = bacc.Bacc("TRN2", target_bir_lowering=False, debug=False,
                   num_devices=NCORES)

    # ---- DRAM I/O ----
    xb = nc.dram_tensor("xb", [S, D], F32R, kind="ExternalInput")
    # wq | wk | wv, each pre-chunked to [128, NDC*FPC]
    wqkv = nc.dram_tensor("wqkv", [128, 3 * NDC * FPC], F32R,
                          kind="ExternalInput")
    wp = nc.dram_tensor("wp", [128, 2 * D], F32R, kind="ExternalInput")
    bqk = nc.dram_tensor("bqk", [128, 4], F32, kind="ExternalInput")
    bv = nc.dram_tensor("bv", [1, FPC], F32R, kind="ExternalInput")
    ident_r = nc.dram_tensor("ident_r", [128, 128], F32R, kind="ExternalInput")
    madd = nc.dram_tensor("madd", [128, 384], F32R, kind="ExternalInput")
    onesr = nc.dram_tensor("onesr", [1, 128], F32R, kind="ExternalInput")

    outp = nc.dram_tensor("outp", [S, D], F32, kind="ExternalOutput")
    pk = nc.dram_tensor("pk", [HPC, S, HD], F32, kind="ExternalOutput")
    pv = nc.dram_tensor("pv", [HPC, S, HD], F32, kind="ExternalOutput")

    Exp = mybir.ActivationFunctionType.Exp
    Ident = mybir.ActivationFunctionType.Identity

    with TileContext(nc) as tc:
        with tc.tile_pool(name="const", bufs=1) as cpool, \
             tc.tile_pool(name="qkv", bufs=1) as qkv, \
             tc.tile_pool(name="ot", bufs=1) as otp:

            # identity first (x transposes depend on it)
            idr_sb = cpool.tile([128, 128], F32R, tag="idr")
            nc.sync.dma_start(out=idr_sb[:], in_=ident_r[:])

            # persistent activations
            qt_sb = [qkv.tile([128, S], F32R, tag=f"qt{i}", name=f"qt{i}")
                     for i in range(2)]
            kt_sb = [qkv.tile([128, S], F32R, tag=f"kt{i}", name=f"kt{i}")
                     for i in range(2)]
            v_sb = qkv.tile([128, NQT * FPC], F32, tag="v")
            # V|ones fp16 per head: head hl block [128, NQT*128];
            # cols 64:128 of each chunk stay 1.0 (denominator trick)
            vb_sb = qkv.tile([128, HPC * NQT * 128], F16, tag="vb")
            ot_sb = [otp.tile([128, S], F32R, tag=f"ot{i}", name=f"ot{i}")
                     for i in range(2)]

            nc.gpsimd.memset(vb_sb[:], 1.0)

            # ---- stage 1+2: x load/transpose + QKV projection ----
            with tc.tile_pool(name="xt", bufs=1) as xtp, \
                 tc.tile_pool(name="xload", bufs=5) as xlp, \
                 tc.tile_pool(name="tp_ps", bufs=2, space="PSUM") as tpps, \
                 tc.tile_pool(name="qk_ps", bufs=4, space="PSUM") as qkps, \
                 tc.tile_pool(name="vp_ps", bufs=2, space="PSUM") as vpps:

                xt_sb = xtp.tile([128, NDC * S], F32R, tag="xt")
                xt3 = xt_sb[:].rearrange("p (dc s) -> p dc s", dc=NDC)

                # prefetch the first S-group's x tiles ahead of the bulky
                # weight transfer so PE transposes start immediately
                x_pre = {}
                for st in range(4):
                    x_t = xlp.tile([128, D], F32R, tag="x", name=f"x{st}")
                    nc.sync.dma_start(out=x_t[:],
                                      in_=xb[st * 128:(st + 1) * 128, :])
                    x_pre[st] = x_t

                # weights next in emission (small, DMA queue drains while
                # x tiles stream)
                wqkv_sb = cpool.tile([128, 3 * NDC * FPC], F32R, tag="wqkv")
                nc.sync.dma_start(out=wqkv_sb[:], in_=wqkv[:])
                wq_sb = wqkv_sb[:, 0:NDC * FPC]
                wk_sb = wqkv_sb[:, NDC * FPC:2 * NDC * FPC]
                wv_sb = wqkv_sb[:, 2 * NDC * FPC:3 * NDC * FPC]
                bqk_sb = cpool.tile([128, 4], F32, tag="bqk")
                nc.sync.dma_start(out=bqk_sb[:], in_=bqk[:])
                bv_sb = cpool.tile([1, FPC], F32R, tag="bv")
                nc.sync.dma_start(out=bv_sb[:], in_=bv[:])
                madd_sb = cpool.tile([128, 384], F32R, tag="madd")
                nc.sync.dma_start(out=madd_sb[:], in_=madd[:])
                on_sb = cpool.tile([1, 128], F32R, tag="on")
                nc.sync.dma_start(out=on_sb[:], in_=onesr[:])

                # interleave x load/transpose with QKV so PE never waits
                # for the full x transfer (PE streams are in-order)
                for sc in range(4):
                    for st in range(4 * sc, 4 * sc + 4):
                        if st in x_pre:
                            x_t = x_pre.pop(st)
                        else:
                            x_t = xlp.tile([128, D], F32R, tag="x",
                                           name=f"x{st}")
                            nc.sync.dma_start(
                                out=x_t[:],
                                in_=xb[st * 128:(st + 1) * 128, :])
                        for dg in range(2):
                            ps = tpps.tile([128, 512], F32R, tag="tp",
                                           name=f"tp{st}_{dg}")
                            for j in range(4):
                                dc = dg * 4 + j
                                nc.tensor.transpose(
                                    ps[:, j * 128:(j + 1) * 128],
                                    x_t[:, dc * 128:(dc + 1) * 128],
                                    idr_sb[:])
                            dst = xt3[:, dg * 4:(dg + 1) * 4,
                                      st * 128:st * 128 + 128]
                            srcp = ps[:].rearrange("p (j s) -> p j s", j=4)
                            if (st + dg) % 2 == 0:
                                nc.vector.tensor_copy(dst, srcp)
                            else:
                                nc.scalar.copy(dst, srcp)

                    # Q^T / K^T columns for this 512-wide S group.
                    # Q evicts on ACT (+bias), K evicts on DVE (+bias).
                    for wi, (w_sb, dstt) in enumerate(((wq_sb, qt_sb),
                                                       (wk_sb, kt_sb))):
                        for ft in range(2):
                            psq = qkps.tile([128, 512], F32, tag="qk",
                                            name=f"qkps{sc}_{wi}{ft}")
                            for dc in range(NDC):
                                lhsT = w_sb[:, dc * FPC + ft * 128:
                                            dc * FPC + ft * 128 + 128]
                                nc.tensor.matmul(
                                    psq[:], lhsT,
                                    xt_sb[:, dc * S + sc * 512:
                                          dc * S + sc * 512 + 512],
                                    start=(dc == 0), stop=(dc == NDC - 1))
                            bias_ap = bqk_sb[:, 2 * wi + ft: 2 * wi + ft + 1]
                            dslc = dstt[ft][:, sc * 512:(sc + 1) * 512]
                            if wi == 0:
                                nc.scalar.activation(dslc, psq[:], Ident,
                                                     bias=bias_ap)
                            else:
                                nc.vector.tensor_scalar_add(dslc, psq[:],
                                                            bias_ap)

                    # V natural + fused bias for these 4 S-tiles
                    for st in range(4 * sc, 4 * sc + 4):
                        vp = vpps.tile([128, FPC], F32, tag="vp",
                                       name=f"vp{st}")
                        for dc in range(NDC):
                            nc.tensor.matmul(
                                vp[:],
                                xt_sb[:, dc * S + st * 128:
                                      dc * S + st * 128 + 128],
                                wv_sb[:, dc * FPC:(dc + 1) * FPC],
                                start=(dc == 0), stop=False)
                        nc.tensor.matmul(vp[:], on_sb[:], bv_sb[:],
                                         start=False, stop=True)
                        nc.scalar.copy(v_sb[:, st * FPC:(st + 1) * FPC],
                                       vp[:])
                        srcv = vp[:].rearrange("p (hl c) -> p hl c", hl=HPC)
                        dst3 = vb_sb[:].rearrange(
                            "p (hl t) -> p hl t", hl=HPC)[
                            :, :, st * 128: st * 128 + HD]
                        if st % 2 == 0:
                            nc.vector.tensor_copy(dst3, srcv)
                        else:
                            nc.scalar.copy(dst3, srcv)

            # pools that reuse the space freed by xt: wp, K-natural
            # collector, attention transients, out staging
            with tc.tile_pool(name="late", bufs=1) as late, \
                 tc.tile_pool(name="pt", bufs=4) as ptp, \
                 tc.tile_pool(name="rs", bufs=4) as rsp, \
                 tc.tile_pool(name="osb", bufs=2) as osbp:

                wp_sb = late.tile([128, 2 * D], F32R, tag="wp")
                nc.sync.dma_start(out=wp_sb[:], in_=wp[:])
                kn_sb = late.tile([128, 2 * NQT * 128], F32, tag="kn")

                # ---- stage 3: K natural + present outputs ----
                with tc.tile_pool(name="kn_ps", bufs=3, space="PSUM") as knps:
                    for ft in range(2):
                        for st in range(NQT):
                            kp = knps.tile([128, 128], F32R, tag="knp")
                            nc.tensor.transpose(
                                kp[:], kt_sb[ft][:, st * 128:(st + 1) * 128],
                                idr_sb[:])
                            dstk = kn_sb[:, ft * S + st * 128:
                                         ft * S + st * 128 + 128]
                            if st % 2 == 0:
                                nc.scalar.copy(dstk, kp[:].bitcast(F32))
                            else:
                                nc.vector.tensor_copy(dstk, kp[:].bitcast(F32))
                    # one DMA per head
                    kn4 = kn_sb[:].rearrange(
                        "p (ft st h d) -> p ft st h d", ft=2, st=NQT, h=2)
                    for ft in range(2):
                        for h2 in range(2):
                            nc.sync.dma_start(
                                out=pk[2 * ft + h2, :, :].rearrange(
                                    "(st p) d -> p st d", p=128),
                                in_=kn4[:, ft, :, h2, :])
                    v4 = v_sb[:].rearrange(
                        "p (st hl d) -> p st hl d", st=NQT, hl=HPC)
                    for hl in range(HPC):
                        nc.sync.dma_start(
                            out=pv[hl, :, :].rearrange(
                                "(st p) d -> p st d", p=128),
                            in_=v4[:, :, hl, :])

                # ---- stage 4: attention per head (software-pipelined).
                # The last head also interleaves the out-projection so PE
                # fills exp-latency stalls with useful work. ----
                def attention_head(hl, stps, ops, norm_lag, per_chunk_hook):
                    ft, po = hl // 2, (hl % 2) * 64
                    kth = kt_sb[ft]
                    qth = qt_sb[ft]
                    oth = ot_sb[ft]
                    vbh = vb_sb[:, hl * NQT * 128:(hl + 1) * NQT * 128]

                    pts = [None] * NQT
                    osums = [None] * NQT

                    def qk_exp_mask(c):
                        qw = min(384, S - c * 128)
                        sp = stps.tile([128, 384], F32, tag="sp",
                                       name=f"sp{hl}_{c}")
                        nc.tensor.matmul(
                            sp[:, :qw],
                            kth[po:po + 64, c * 128:(c + 1) * 128],
                            qth[po:po + 64, c * 128:c * 128 + qw],
                            start=True, stop=False)
                        # additive mask (-1e30 off-window) via I.T @ madd
                        nc.tensor.matmul(
                            sp[:, :qw], idr_sb[:], madd_sb[:, :qw],
                            start=False, stop=True)
                        pt = ptp.tile([128, 384], F16, tag="pt",
                                      name=f"pt{hl}_{c}")
                        nc.scalar.activation(pt[:, :qw], sp[:, :qw], Exp)
                        pts[c] = pt

                    def osum(c):
                        if osums[c] is None:
                            osums[c] = ops.tile([128, 128], F32, tag="o",
                                                name=f"o{hl}_{c}")
                        return osums[c]

                    def normalize(c):
                        o_cur = osums[c]
                        rec = rsp.tile([64, 128], F32, tag="rec",
                                       name=f"rec{hl}_{c}")
                        nc.vector.reciprocal(rec[:], o_cur[64:128, :])
                        nc.vector.tensor_mul(
                            oth[po:po + 64, c * 128:(c + 1) * 128],
                            o_cur[0:64, :], rec[:])
                        osums[c] = None

                    qk_exp_mask(0)
                    qk_exp_mask(1)
                    for c in range(NQT):
                        if c + 2 < NQT:
                            qk_exp_mask(c + 2)
                        pt = pts[c]
                        qw = min(384, S - c * 128)
                        lhsT = vbh[:, c * 128:(c + 1) * 128]
                        nc.tensor.matmul(osum(c)[:], lhsT, pt[:, 0:128],
                                         start=(c == 0), stop=True)
                        if c + 1 < NQT:
                            nc.tensor.matmul(osum(c + 1)[:], lhsT,
                                             pt[:, 128:256],
                                             start=(c == 0), stop=False)
                        if c + 2 < NQT and qw > 256:
                            nc.tensor.matmul(osum(c + 2)[:], lhsT,
                                             pt[:, 256:384],
                                             start=True, stop=False)
                        # normalization lags the AV matmuls so the DVE
                        # stream never round-trips against PE
                        if c >= norm_lag:
                            normalize(c - norm_lag)
                            if per_chunk_hook is not None:
                                per_chunk_hook(c - norm_lag)
                        pts[c] = None
                    for c in range(NQT - norm_lag, NQT):
                        normalize(c)
                        if per_chunk_hook is not None:
                            per_chunk_hook(c)

                osb_state = {}

                def outproj_tile(st, opps):
                    # called once per q-tile st (in order) after all heads
                    # normalized it
                    st2, i = st // 2, st % 2
                    if i == 0:
                        osb_state["t"] = osbp.tile([128, 2 * D], F32,
                                                   tag="osb",
                                                   name=f"osb{st2}")
                    o_t = osb_state["t"]
                    for half in range(2):
                        op = opps.tile([128, 512], F32, tag="op",
                                       name=f"op{st}_{half}")
                        nc.tensor.matmul(
                            op[:], ot_sb[0][:, st * 128:(st + 1) * 128],
                            wp_sb[:, half * 512: half * 512 + 512],
                            start=True, stop=False)
                        nc.tensor.matmul(
                            op[:], ot_sb[1][:, st * 128:(st + 1) * 128],
                            wp_sb[:, D + half * 512: D + half * 512 + 512],
                            start=False, stop=True)
                        dsl = o_t[:, i * D + half * 512:
                                  i * D + (half + 1) * 512]
                        if half == 0:
                            nc.scalar.copy(dsl, op[:])
                        else:
                            nc.vector.tensor_copy(dsl, op[:])
                    if i == 1:
                        nc.sync.dma_start(
                            out=outp[st2 * 256:(st2 + 1) * 256, :].rearrange(
                                "(j p) d -> p j d", p=128),
                            in_=o_t[:].rearrange("p (j d) -> p j d", j=2))

                with tc.tile_pool(name="st_ps", bufs=3, space="PSUM") as stps, \
                     tc.tile_pool(name="o_ps", bufs=5, space="PSUM") as ops:
                    for hl in range(HPC - 1):
                        attention_head(hl, stps, ops, 2, None)

                with tc.tile_pool(name="st_ps2", bufs=2, space="PSUM") as stps, \
                     tc.tile_pool(name="o_ps2", bufs=4, space="PSUM") as ops, \
                     tc.tile_pool(name="op_ps", bufs=2, space="PSUM") as opps:
                    attention_head(HPC - 1, stps, ops, 1,
                                   lambda st: outproj_tile(st, opps))

    nc.compile()
    return nc


def _prep_in_maps(x, w_attn, b_attn, w_proj):
    """Per-core input dicts (host-side sharding + layout prep)."""
    x = np.ascontiguousarray(np.asarray(x, dtype=np.float32))
    w_attn = np.asarray(w_attn, dtype=np.float32)
    b_attn = np.asarray(b_attn, dtype=np.float32)
    w_proj = np.asarray(w_proj, dtype=np.float32)

    ident = np.eye(128, dtype=np.float32)
    ql = np.arange(128)[None, :]
    kl = np.arange(128)[:, None]
    neg = np.float32(-1e30)
    madd = np.concatenate(
        [np.where(ql >= kl, np.float32(0), neg),
         np.zeros((128, 128), np.float32),
         np.where(ql < kl, np.float32(0), neg)], axis=1).astype(np.float32)
    onesr = np.ones((1, 128), dtype=np.float32)

    def chunk_w(w_cols):  # [D, FPC] -> [128, NDC*FPC]
        return w_cols.reshape(NDC, 128, FPC).transpose(1, 0, 2).reshape(
            128, NDC * FPC)

    in_maps = []
    for core in range(NCORES):
        b, hg = core // 4, core % 4
        cols = slice(hg * FPC, (hg + 1) * FPC)
        kcols = slice(D + hg * FPC, D + (hg + 1) * FPC)
        vcols = slice(2 * D + hg * FPC, 2 * D + (hg + 1) * FPC)
        rows = slice(hg * FPC, (hg + 1) * FPC)
        wqkv = np.concatenate(
            [chunk_w(w_attn[:, cols] * np.float32(SCALE)),
             chunk_w(w_attn[:, kcols]),
             chunk_w(w_attn[:, vcols])], axis=1)
        bqk = np.stack(
            [(b_attn[cols] * np.float32(SCALE)).reshape(2, 128)[0],
             (b_attn[cols] * np.float32(SCALE)).reshape(2, 128)[1],
             b_attn[kcols].reshape(2, 128)[0],
             b_attn[kcols].reshape(2, 128)[1]], axis=1)
        in_maps.append({
            "xb": x[b],
            "wqkv": np.ascontiguousarray(wqkv),
            "wp": np.ascontiguousarray(
                w_proj[rows, :].reshape(2, 128, D).transpose(1, 0, 2).reshape(
                    128, 2 * D)),
            "bqk": np.ascontiguousarray(bqk),
            "bv": b_attn[vcols].reshape(1, FPC).copy(),
            "ident_r": ident,
            "madd": madd,
            "onesr": onesr,
        })
    return in_maps


def kernel(x, w_attn, b_attn, w_proj, b_proj):
    if "nc" not in _CACHE:
        _CACHE["nc"] = _build_program()
    nc = _CACHE["nc"]

    in_maps = _prep_in_maps(x, w_attn, b_attn, w_proj)
    res = run_bass_kernel_spmd(nc, in_maps, core_ids=list(range(NCORES)))

    b_proj = np.asarray(b_proj, dtype=np.float32)
    out = np.zeros((B, S, D), dtype=np.float32)
    present = np.zeros((B, 2, N_HEAD, S, HD), dtype=np.float32)
    for core in range(NCORES):
        b, hg = core // 4, core % 4
        r = res.results[core]
        out[b] += r["outp"]
        present[b, 0, hg * HPC:(hg + 1) * HPC] = r["pk"]
        present[b, 1, hg * HPC:(hg + 1) * HPC] = r["pv"]
    out += b_proj
    return out, present


# revision 13
# speedup vs baseline: 1.1378x; 1.1282x over previous
"""Trainium2 Bass kernel for sliding-window causal attention block.

Reference computation (B=2, S=2048, D=1024, H=16, hd=64, WINDOW=256):
    c = x @ w_attn + b_attn ; q,k,v = split(c)
    present = stack([k, v]) as [B,2,H,S,hd]
    att = softmax(mask(q k^T / sqrt(hd))) @ v
    out = att @ w_proj + b_proj

Sharding: 8 cores = 2 batches x 4 head-groups (4 heads each).
Per core: QKV projection for its 256 q/k/v features (column-sharded
w_attn), attention for its 4 heads, and a partial out-projection
(row-sharded w_proj). Host sums the 4 partials per batch and adds
b_proj exactly.

Layout strategy on-core:
  x[b] is PE-transposed once to xT [D, S] (f32r, 4 transposes batched
  per PSUM bank, eviction split across DVE/ACT).
  Q^T, K^T produced feature-major [feat, S] (f32r), V natural [S, feat].
  Scores are computed directly transposed: S^T[kpos, q] tile per
  128-kpos chunk covering its 384 valid q columns (window 256 spans 3
  q-tiles), so softmax needs no P transposes. Masks: causal triangle on
  the left third, all-valid middle, anti-causal triangle right.
  exp on ACT -> P^T fp16; AV matmul uses V|ones fp16 where columns
  64:128 are all ones, so the softmax denominator lands replicated on
  PSUM partitions 64:128 -> 64-lane reciprocal + multiply on DVE.
  O^T f32r; out-projection back to natural [S, D] layout.
  DMAs are batched aggressively (the SP sequencer pays ~0.6us per DMA
  instruction): 8 input DMAs for x, 1 for all of wq/wk/wv, single
  per-head DMAs for the present k/v outputs, 2-S-tile DMAs for out.
"""

import sys

sys.path.insert(0, "/opt/trn_rl_repo")

import numpy as np

import concourse.bass as bass  # noqa: F401  (bass must import before bacc)
import concourse.mybir as mybir
from concourse import bacc
from concourse.tile import TileContext
from concourse.bass_utils import run_bass_kernel_spmd

F32 = mybir.dt.float32
F32R = mybir.dt.float32r
F16 = mybir.dt.float16

B, S, D = 2, 2048, 1024
N_HEAD = 16
HD = 64
WINDOW = 256
NCORES = 8
HPC = N_HEAD // 4  # heads per core = 4
FPC = HPC * HD  # features per core = 256
NQT = S // 128  # 16 q/kpos tiles
NDC = D // 128  # 8 contraction chunks
SCALE = 1.0 / np.sqrt(HD)

_CACHE = {}


def _build_program():
    nc = bacc.Bacc("TRN2", target_bir_lowering=False, debug=False,
                   num_devices=NCORES)

    # ---- DRAM I/O ----
    xb = nc.dram_tensor("xb", [S, D], F32R, kind="ExternalInput")
    # wq | wk | wv, each pre-chunked to [128, NDC*FPC]
    wqkv = nc.dram_tensor("wqkv", [128, 3 * NDC * FPC], F32R,
                          kind="ExternalInput")
    wp = nc.dram_tensor("wp", [128, 2 * D], F32R, kind="ExternalInput")
    bqk = nc.dram_tensor("bqk", [128, 4], F32, kind="ExternalInput")
    bv = nc.dram_tensor("bv", [1, FPC], F32R, kind="ExternalInput")
    ident_r = nc.dram_tensor("ident_r", [128, 128], F32R, kind="ExternalInput")
    madd = nc.dram_tensor("madd", [128, 384], F32R, kind="ExternalInput")
    onesr = nc.dram_tensor("onesr", [1, 128], F32R, kind="ExternalInput")

    outp = nc.dram_tensor("outp", [S, D], F32, kind="ExternalOutput")
    pk = nc.dram_tensor("pk", [HPC, S, HD], F32, kind="ExternalOutput")
    pv = nc.dram_tensor("pv", [HPC, S, HD], F32, kind="ExternalOutput")

    Exp = mybir.ActivationFunctionType.Exp
    Ident = mybir.ActivationFunctionType.Identity

    with TileContext(nc) as tc:
        with tc.tile_pool(name="const", bufs=1) as cpool, \
             tc.tile_pool(name="qkv", bufs=1) as qkv, \
             tc.tile_pool(name="ot", bufs=1) as otp:

            # identity first (x transposes depend on it)
            idr_sb = cpool.tile([128, 128], F32R, tag="idr")
            nc.sync.dma_start(out=idr_sb[:], in_=ident_r[:])

            # persistent activations
            qt_sb = [qkv.tile([128, S], F32R, tag=f"qt{i}", name=f"qt{i}")
                     for i in range(2)]
            kt_sb = [qkv.tile([128, S], F32R, tag=f"kt{i}", name=f"kt{i}")
                     for i in range(2)]
            v_sb = qkv.tile([128, NQT * FPC], F32, tag="v")
            # V|ones fp16 per head: head hl block [128, NQT*128];
            # cols 64:128 of each chunk stay 1.0 (denominator trick)
            vb_sb = qkv.tile([128, HPC * NQT * 128], F16, tag="vb")
            ot_sb = [otp.tile([128, S], F32R, tag=f"ot{i}", name=f"ot{i}")
                     for i in range(2)]

            nc.gpsimd.memset(vb_sb[:], 1.0)

            # ---- stage 1+2: x load/transpose + QKV projection ----
            with tc.tile_pool(name="xt", bufs=1) as xtp, \
                 tc.tile_pool(name="xload", bufs=3) as xlp, \
                 tc.tile_pool(name="tp_ps", bufs=2, space="PSUM") as tpps, \
                 tc.tile_pool(name="qk_ps", bufs=4, space="PSUM") as qkps, \
                 tc.tile_pool(name="vp_ps", bufs=2, space="PSUM") as vpps:

                xt_sb = xtp.tile([128, NDC * S], F32R, tag="xt")
                xt3 = xt_sb[:].rearrange("p (dc s) -> p dc s", dc=NDC)

                # weights next in emission (small, DMA queue drains while
                # x tiles stream)
                wqkv_sb = cpool.tile([128, 3 * NDC * FPC], F32R, tag="wqkv")
                nc.sync.dma_start(out=wqkv_sb[:], in_=wqkv[:])
                wq_sb = wqkv_sb[:, 0:NDC * FPC]
                wk_sb = wqkv_sb[:, NDC * FPC:2 * NDC * FPC]
                wv_sb = wqkv_sb[:, 2 * NDC * FPC:3 * NDC * FPC]
                bqk_sb = cpool.tile([128, 4], F32, tag="bqk")
                nc.sync.dma_start(out=bqk_sb[:], in_=bqk[:])
                bv_sb = cpool.tile([1, FPC], F32R, tag="bv")
                nc.sync.dma_start(out=bv_sb[:], in_=bv[:])
                madd_sb = cpool.tile([128, 384], F32R, tag="madd")
                nc.sync.dma_start(out=madd_sb[:], in_=madd[:])
                on_sb = cpool.tile([1, 128], F32R, tag="on")
                nc.sync.dma_start(out=on_sb[:], in_=onesr[:])

                # interleave x load/transpose with QKV so PE never waits
                # for the full x transfer (PE streams are in-order)
                for sc in range(4):
                    for st in range(4 * sc, 4 * sc + 4):
                        x_t = xlp.tile([128, D], F32R, tag="x",
                                       name=f"x{st}")
                        nc.sync.dma_start(
                            out=x_t[:],
                            in_=xb[st * 128:(st + 1) * 128, :])
                        for dg in range(2):
                            ps = tpps.tile([128, 512], F32R, tag="tp",
                                           name=f"tp{st}_{dg}")
                            for j in range(4):
                                dc = dg * 4 + j
                                nc.tensor.transpose(
                                    ps[:, j * 128:(j + 1) * 128],
                                    x_t[:, dc * 128:(dc + 1) * 128],
                                    idr_sb[:])
                            dst = xt3[:, dg * 4:(dg + 1) * 4,
                                      st * 128:st * 128 + 128]
                            srcp = ps[:].rearrange("p (j s) -> p j s", j=4)
                            if (st + dg) % 2 == 0:
                                nc.vector.tensor_copy(dst, srcp)
                            else:
                                nc.scalar.copy(dst, srcp)

                    # Q^T / K^T columns for this 512-wide S group.
                    # Q evicts on ACT (+bias), K evicts on DVE (+bias).
                    for wi, (w_sb, dstt) in enumerate(((wq_sb, qt_sb),
                                                       (wk_sb, kt_sb))):
                        for ft in range(2):
                            psq = qkps.tile([128, 512], F32, tag="qk",
                                            name=f"qkps{sc}_{wi}{ft}")
                            for dc in range(NDC):
                                lhsT = w_sb[:, dc * FPC + ft * 128:
                                            dc * FPC + ft * 128 + 128]
                                nc.tensor.matmul(
                                    psq[:], lhsT,
                                    xt_sb[:, dc * S + sc * 512:
                                          dc * S + sc * 512 + 512],
                                    start=(dc == 0), stop=(dc == NDC - 1))
                            bias_ap = bqk_sb[:, 2 * wi + ft: 2 * wi + ft + 1]
                            dslc = dstt[ft][:, sc * 512:(sc + 1) * 512]
                            if wi == 0:
                                nc.scalar.activation(dslc, psq[:], Ident,
                                                     bias=bias_ap)
                            else:
                                nc.vector.tensor_scalar_add(dslc, psq[:],
                                                            bias_ap)

                    # V natural + fused bias for these 4 S-tiles
                    for st in range(4 * sc, 4 * sc + 4):
                        vp = vpps.tile([128, FPC], F32, tag="vp",
                                       name=f"vp{st}")
                        for dc in range(NDC):
                            nc.tensor.matmul(
                                vp[:],
                                xt_sb[:, dc * S + st * 128:
                                      dc * S + st * 128 + 128],
                                wv_sb[:, dc * FPC:(dc + 1) * FPC],
                                start=(dc == 0), stop=False)
                        nc.tensor.matmul(vp[:], on_sb[:], bv_sb[:],
                                         start=False, stop=True)
                        nc.scalar.copy(v_sb[:, st * FPC:(st + 1) * FPC],
                                       vp[:])
                        srcv = vp[:].rearrange("p (hl c) -> p hl c", hl=HPC)
                        dst3 = vb_sb[:].rearrange(
                            "p (hl t) -> p hl t", hl=HPC)[
                            :, :, st * 128: st * 128 + HD]
                        if st % 2 == 0:
                            nc.vector.tensor_copy(dst3, srcv)
                        else:
                            nc.scalar.copy(dst3, srcv)

            # pools that reuse the space freed by xt: wp, K-natural
            # collector, attention transients, out staging
            with tc.tile_pool(name="late", bufs=1) as late, \
                 tc.tile_pool(name="pt", bufs=4) as ptp, \
                 tc.tile_pool(name="rs", bufs=4) as rsp, \
                 tc.tile_pool(name="osb", bufs=2) as osbp:

                wp_sb = late.tile([128, 2 * D], F32R, tag="wp")
                nc.sync.dma_start(out=wp_sb[:], in_=wp[:])
                kn_sb = late.tile([128, 2 * NQT * 128], F32, tag="kn")

                # ---- stage 3: K natural + present outputs ----
                with tc.tile_pool(name="kn_ps", bufs=3, space="PSUM") as knps:
                    for ft in range(2):
                        for st in range(NQT):
                            kp = knps.tile([128, 128], F32R, tag="knp")
                            nc.tensor.transpose(
                                kp[:], kt_sb[ft][:, st * 128:(st + 1) * 128],
                                idr_sb[:])
                            dstk = kn_sb[:, ft * S + st * 128:
                                         ft * S + st * 128 + 128]
                            if st % 2 == 0:
                                nc.scalar.copy(dstk, kp[:].bitcast(F32))
                            else:
                                nc.vector.tensor_copy(dstk, kp[:].bitcast(F32))
                    # one DMA per head
                    kn4 = kn_sb[:].rearrange(
                        "p (ft st h d) -> p ft st h d", ft=2, st=NQT, h=2)
                    for ft in range(2):
                        for h2 in range(2):
                            nc.sync.dma_start(
                                out=pk[2 * ft + h2, :, :].rearrange(
                                    "(st p) d -> p st d", p=128),
                                in_=kn4[:, ft, :, h2, :])
                    v4 = v_sb[:].rearrange(
                        "p (st hl d) -> p st hl d", st=NQT, hl=HPC)
                    for hl in range(HPC):
                        nc.sync.dma_start(
                            out=pv[hl, :, :].rearrange(
                                "(st p) d -> p st d", p=128),
                            in_=v4[:, :, hl, :])

                # ---- stage 4: attention per head (software-pipelined).
                # The last head also interleaves the out-projection so PE
                # fills exp-latency stalls with useful work. ----
                def attention_head(hl, stps, ops, norm_lag, per_chunk_hook):
                    ft, po = hl // 2, (hl % 2) * 64
                    kth = kt_sb[ft]
                    qth = qt_sb[ft]
                    oth = ot_sb[ft]
                    vbh = vb_sb[:, hl * NQT * 128:(hl + 1) * NQT * 128]

                    pts = [None] * NQT
                    osums = [None] * NQT

                    def qk_exp_mask(c):
                        qw = min(384, S - c * 128)
                        sp = stps.tile([128, 384], F32, tag="sp",
                                       name=f"sp{hl}_{c}")
                        nc.tensor.matmul(
                            sp[:, :qw],
                            kth[po:po + 64, c * 128:(c + 1) * 128],
                            qth[po:po + 64, c * 128:c * 128 + qw],
                            start=True, stop=False)
                        # additive mask (-1e30 off-window) via I.T @ madd
                        nc.tensor.matmul(
                            sp[:, :qw], idr_sb[:], madd_sb[:, :qw],
                            start=False, stop=True)
                        pt = ptp.tile([128, 384], F16, tag="pt",
                                      name=f"pt{hl}_{c}")
                        nc.scalar.activation(pt[:, :qw], sp[:, :qw], Exp)
                        pts[c] = pt

                    def osum(c):
                        if osums[c] is None:
                            osums[c] = ops.tile([128, 128], F32, tag="o",
                                                name=f"o{hl}_{c}")
                        return osums[c]

                    def normalize(c):
                        o_cur = osums[c]
                        rec = rsp.tile([64, 128], F32, tag="rec",
                                       name=f"rec{hl}_{c}")
                        nc.vector.reciprocal(rec[:], o_cur[64:128, :])
                        nc.vector.tensor_mul(
                            oth[po:po + 64, c * 128:(c + 1) * 128],
                            o_cur[0:64, :], rec[:])
                        osums[c] = None

                    qk_exp_mask(0)
                    qk_exp_mask(1)
                    for c in range(NQT):
                        if c + 2 < NQT:
                            qk_exp_mask(c + 2)
                        pt = pts[c]
                        qw = min(384, S - c * 128)
                        lhsT = vbh[:, c * 128:(c + 1) * 128]
                        nc.tensor.matmul(osum(c)[:], lhsT, pt[:, 0:128],
                                         start=(c == 0), stop=True)
                        if c + 1 < NQT:
                            nc.tensor.matmul(osum(c + 1)[:], lhsT,
                                             pt[:, 128:256],
                                             start=(c == 0), stop=False)
                        if c + 2 < NQT and qw > 256:
                            nc.tensor.matmul(osum(c + 2)[:], lhsT,
                                             pt[:, 256:384],
                                             start=True, stop=False)
                        # normalization lags the AV matmuls so the DVE
                        # stream never round-trips against PE
                        if c >= norm_lag:
                            normalize(c - norm_lag)
                            if per_chunk_hook is not None:
                                per_chunk_hook(c - norm_lag)
                        pts[c] = None
                    for c in range(NQT - norm_lag, NQT):
                        normalize(c)
                        if per_chunk_hook is not None:
                            per_chunk_hook(c)

                osb_state = {}

                def outproj_tile(st, opps):
                    # called once per q-tile st (in order) after all heads
                    # normalized it
                    st2, i = st // 2, st % 2
                    if i == 0:
                        osb_state["t"] = osbp.tile([128, 2 * D], F32,
                                                   tag="osb",
                                                   name=f"osb{st2}")
                    o_t = osb_state["t"]
                    for half in range(2):
                        op = opps.tile([128, 512], F32, tag="op",
                                       name=f"op{st}_{half}")
                        nc.tensor.matmul(
                            op[:], ot_sb[0][:, st * 128:(st + 1) * 128],
                            wp_sb[:, half * 512: half * 512 + 512],
                            start=True, stop=False)
                        nc.tensor.matmul(
                            op[:], ot_sb[1][:, st * 128:(st + 1) * 128],
                            wp_sb[:, D + half * 512: D + half * 512 + 512],
                            start=False, stop=True)
                        dsl = o_t[:, i * D + half * 512:
                                  i * D + (half + 1) * 512]
                        if half == 0:
                            nc.scalar.copy(dsl, op[:])
                        else:
                            nc.vector.tensor_copy(dsl, op[:])
                    if i == 1:
                        nc.sync.dma_start(
                            out=outp[st2 * 256:(st2 + 1) * 256, :].rearrange(
                                "(j p) d -> p j d", p=128),
                            in_=o_t[:].rearrange("p (j d) -> p j d", j=2))

                with tc.tile_pool(name="st_ps", bufs=3, space="PSUM") as stps, \
                     tc.tile_pool(name="o_ps", bufs=5, space="PSUM") as ops:
                    for hl in range(HPC - 1):
                        attention_head(hl, stps, ops, 2, None)

                with tc.tile_pool(name="st_ps2", bufs=2, space="PSUM") as stps, \
                     tc.tile_pool(name="o_ps2", bufs=4, space="PSUM") as ops, \
                     tc.tile_pool(name="op_ps", bufs=2, space="PSUM") as opps:
                    attention_head(HPC - 1, stps, ops, 1,
                                   lambda st: outproj_tile(st, opps))

    nc.compile()
    return nc


def _prep_in_maps(x, w_attn, b_attn, w_proj):
    """Per-core input dicts (host-side sharding + layout prep)."""
    x = np.ascontiguousarray(np.asarray(x, dtype=np.float32))
    w_attn = np.asarray(w_attn, dtype=np.float32)
    b_attn = np.asarray(b_attn, dtype=np.float32)
    w_proj = np.asarray(w_proj, dtype=np.float32)

    ident = np.eye(128, dtype=np.float32)
    ql = np.arange(128)[None, :]
    kl = np.arange(128)[:, None]
    neg = np.float32(-1e30)
    madd = np.concatenate(
        [np.where(ql >= kl, np.float32(0), neg),
         np.zeros((128, 128), np.float32),
         np.where(ql < kl, np.float32(0), neg)], axis=1).astype(np.float32)
    onesr = np.ones((1, 128), dtype=np.float32)

    def chunk_w(w_cols):  # [D, FPC] -> [128, NDC*FPC]
        return w_cols.reshape(NDC, 128, FPC).transpose(1, 0, 2).reshape(
            128, NDC * FPC)

    in_maps = []
    for core in range(NCORES):
        b, hg = core // 4, core % 4
        cols = slice(hg * FPC, (hg + 1) * FPC)
        kcols = slice(D + hg * FPC, D + (hg + 1) * FPC)
        vcols = slice(2 * D + hg * FPC, 2 * D + (hg + 1) * FPC)
        rows = slice(hg * FPC, (hg + 1) * FPC)
        wqkv = np.concatenate(
            [chunk_w(w_attn[:, cols] * np.float32(SCALE)),
             chunk_w(w_attn[:, kcols]),
             chunk_w(w_attn[:, vcols])], axis=1)
        bqk = np.stack(
            [(b_attn[cols] * np.float32(SCALE)).reshape(2, 128)[0],
             (b_attn[cols] * np.float32(SCALE)).reshape(2, 128)[1],
             b_attn[kcols].reshape(2, 128)[0],
             b_attn[kcols].reshape(2, 128)[1]], axis=1)
        in_maps.append({
            "xb": x[b],
            "wqkv": np.ascontiguousarray(wqkv),
            "wp": np.ascontiguousarray(
                w_proj[rows, :].reshape(2, 128, D).transpose(1, 0, 2).reshape(
                    128, 2 * D)),
            "bqk": np.ascontiguousarray(bqk),
            "bv": b_attn[vcols].reshape(1, FPC).copy(),
            "ident_r": ident,
            "madd": madd,
            "onesr": onesr,
        })
    return in_maps


def kernel(x, w_attn, b_attn, w_proj, b_proj):
    if "nc" not in _CACHE:
        _CACHE["nc"] = _build_program()
    nc = _CACHE["nc"]

    in_maps = _prep_in_maps(x, w_attn, b_attn, w_proj)
    res = run_bass_kernel_spmd(nc, in_maps, core_ids=list(range(NCORES)))

    b_proj = np.asarray(b_proj, dtype=np.float32)
    out = np.zeros((B, S, D), dtype=np.float32)
    present = np.zeros((B, 2, N_HEAD, S, HD), dtype=np.float32)
    for core in range(NCORES):
        b, hg = core // 4, core % 4
        r = res.results[core]
        out[b] += r["outp"]
        present[b, 0, hg * HPC:(hg + 1) * HPC] = r["pk"]
        present[b, 1, hg * HPC:(hg + 1) * HPC] = r["pv"]
    out += b_proj
    return out, present


# revision 14
# speedup vs baseline: 1.1396x; 1.0016x over previous
"""Trainium2 Bass kernel for sliding-window causal attention block.

Reference computation (B=2, S=2048, D=1024, H=16, hd=64, WINDOW=256):
    c = x @ w_attn + b_attn ; q,k,v = split(c)
    present = stack([k, v]) as [B,2,H,S,hd]
    att = softmax(mask(q k^T / sqrt(hd))) @ v
    out = att @ w_proj + b_proj

Sharding: 8 cores = 2 batches x 4 head-groups (4 heads each).
Per core: QKV projection for its 256 q/k/v features (column-sharded
w_attn), attention for its 4 heads, and a partial out-projection
(row-sharded w_proj). Host sums the 4 partials per batch and adds
b_proj exactly.

Layout strategy on-core:
  x[b] is PE-transposed once to xT [D, S] (f32r, 4 transposes batched
  per PSUM bank, eviction split across DVE/ACT).
  Q^T, K^T produced feature-major [feat, S] (f32r), V natural [S, feat].
  Scores are computed directly transposed: S^T[kpos, q] tile per
  128-kpos chunk covering its 384 valid q columns (window 256 spans 3
  q-tiles), so softmax needs no P transposes. Masks: causal triangle on
  the left third, all-valid middle, anti-causal triangle right.
  exp on ACT -> P^T fp16; AV matmul uses V|ones fp16 where columns
  64:128 are all ones, so the softmax denominator lands replicated on
  PSUM partitions 64:128 -> 64-lane reciprocal + multiply on DVE.
  O^T f32r; out-projection back to natural [S, D] layout.
  DMAs are batched aggressively (the SP sequencer pays ~0.6us per DMA
  instruction): 8 input DMAs for x, 1 for all of wq/wk/wv, single
  per-head DMAs for the present k/v outputs, 2-S-tile DMAs for out.
"""

import sys

sys.path.insert(0, "/opt/trn_rl_repo")

import numpy as np

import concourse.bass as bass  # noqa: F401  (bass must import before bacc)
import concourse.mybir as mybir
from concourse import bacc
from concourse.tile import TileContext
from concourse.bass_utils import run_bass_kernel_spmd

F32 = mybir.dt.float32
F32R = mybir.dt.float32r
F16 = mybir.dt.float16

B, S, D = 2, 2048, 1024
N_HEAD = 16
HD = 64
WINDOW = 256
NCORES = 8
HPC = N_HEAD // 4  # heads per core = 4
FPC = HPC * HD  # features per core = 256
NQT = S // 128  # 16 q/kpos tiles
NDC = D // 128  # 8 contraction chunks
SCALE = 1.0 / np.sqrt(HD)

_CACHE = {}


def _build_program():
    nc = bacc.Bacc("TRN2", target_bir_lowering=False, debug=False,
                   num_devices=NCORES)

    # ---- DRAM I/O ----
    xb = nc.dram_tensor("xb", [S, D], F32R, kind="ExternalInput")
    # wq | wk | wv, each pre-chunked to [128, NDC*FPC]
    wqkv = nc.dram_tensor("wqkv", [128, 3 * NDC * FPC], F32R,
                          kind="ExternalInput")
    wp = nc.dram_tensor("wp", [128, 2 * D], F32R, kind="ExternalInput")
    bqk = nc.dram_tensor("bqk", [128, 4], F32, kind="ExternalInput")
    bv = nc.dram_tensor("bv", [1, FPC], F32R, kind="ExternalInput")
    ident_r = nc.dram_tensor("ident_r", [128, 128], F32R, kind="ExternalInput")
    madd = nc.dram_tensor("madd", [128, 384], F32R, kind="ExternalInput")
    onesr = nc.dram_tensor("onesr", [1, 128], F32R, kind="ExternalInput")

    outp = nc.dram_tensor("outp", [S, D], F32, kind="ExternalOutput")
    pk = nc.dram_tensor("pk", [HPC, S, HD], F32, kind="ExternalOutput")
    pv = nc.dram_tensor("pv", [HPC, S, HD], F32, kind="ExternalOutput")

    Exp = mybir.ActivationFunctionType.Exp
    Ident = mybir.ActivationFunctionType.Identity

    with TileContext(nc) as tc:
        with tc.tile_pool(name="const", bufs=1) as cpool, \
             tc.tile_pool(name="qkv", bufs=1) as qkv, \
             tc.tile_pool(name="ot", bufs=1) as otp:

            # identity first (x transposes depend on it)
            idr_sb = cpool.tile([128, 128], F32R, tag="idr")
            nc.sync.dma_start(out=idr_sb[:], in_=ident_r[:])

            # persistent activations
            qt_sb = [qkv.tile([128, S], F32R, tag=f"qt{i}", name=f"qt{i}")
                     for i in range(2)]
            kt_sb = [qkv.tile([128, S], F32R, tag=f"kt{i}", name=f"kt{i}")
                     for i in range(2)]
            v_sb = qkv.tile([128, NQT * FPC], F32, tag="v")
            # V|ones fp16 per head: head hl block [128, NQT*128];
            # cols 64:128 of each chunk stay 1.0 (denominator trick)
            vb_sb = qkv.tile([128, HPC * NQT * 128], F16, tag="vb")
            ot_sb = [otp.tile([128, S], F32R, tag=f"ot{i}", name=f"ot{i}")
                     for i in range(2)]

            nc.gpsimd.memset(vb_sb[:], 1.0)

            # ---- stage 1+2: x load/transpose + QKV projection ----
            with tc.tile_pool(name="xt", bufs=1) as xtp, \
                 tc.tile_pool(name="xload", bufs=3) as xlp, \
                 tc.tile_pool(name="tp_ps", bufs=2, space="PSUM") as tpps, \
                 tc.tile_pool(name="qk_ps", bufs=4, space="PSUM") as qkps, \
                 tc.tile_pool(name="vp_ps", bufs=2, space="PSUM") as vpps:

                xt_sb = xtp.tile([128, NDC * S], F32R, tag="xt")
                xt3 = xt_sb[:].rearrange("p (dc s) -> p dc s", dc=NDC)

                # weights next in emission (small, DMA queue drains while
                # x tiles stream)
                wqkv_sb = cpool.tile([128, 3 * NDC * FPC], F32R, tag="wqkv")
                nc.scalar.dma_start(out=wqkv_sb[:], in_=wqkv[:])
                wq_sb = wqkv_sb[:, 0:NDC * FPC]
                wk_sb = wqkv_sb[:, NDC * FPC:2 * NDC * FPC]
                wv_sb = wqkv_sb[:, 2 * NDC * FPC:3 * NDC * FPC]
                bqk_sb = cpool.tile([128, 4], F32, tag="bqk")
                nc.scalar.dma_start(out=bqk_sb[:], in_=bqk[:])
                bv_sb = cpool.tile([1, FPC], F32R, tag="bv")
                nc.scalar.dma_start(out=bv_sb[:], in_=bv[:])
                madd_sb = cpool.tile([128, 384], F32R, tag="madd")
                nc.scalar.dma_start(out=madd_sb[:], in_=madd[:])
                on_sb = cpool.tile([1, 128], F32R, tag="on")
                nc.scalar.dma_start(out=on_sb[:], in_=onesr[:])

                # interleave x load/transpose with QKV so PE never waits
                # for the full x transfer (PE streams are in-order)
                for sc in range(4):
                    for st in range(4 * sc, 4 * sc + 4):
                        x_t = xlp.tile([128, D], F32R, tag="x",
                                       name=f"x{st}")
                        nc.sync.dma_start(
                            out=x_t[:],
                            in_=xb[st * 128:(st + 1) * 128, :])
                        for dg in range(2):
                            ps = tpps.tile([128, 512], F32R, tag="tp",
                                           name=f"tp{st}_{dg}")
                            for j in range(4):
                                dc = dg * 4 + j
                                nc.tensor.transpose(
                                    ps[:, j * 128:(j + 1) * 128],
                                    x_t[:, dc * 128:(dc + 1) * 128],
                                    idr_sb[:])
                            dst = xt3[:, dg * 4:(dg + 1) * 4,
                                      st * 128:st * 128 + 128]
                            srcp = ps[:].rearrange("p (j s) -> p j s", j=4)
                            if (st + dg) % 2 == 0:
                                nc.vector.tensor_copy(dst, srcp)
                            else:
                                nc.scalar.copy(dst, srcp)

                    # Q^T / K^T columns for this 512-wide S group.
                    # Q evicts on ACT (+bias), K evicts on DVE (+bias).
                    for wi, (w_sb, dstt) in enumerate(((wq_sb, qt_sb),
                                                       (wk_sb, kt_sb))):
                        for ft in range(2):
                            psq = qkps.tile([128, 512], F32, tag="qk",
                                            name=f"qkps{sc}_{wi}{ft}")
                            for dc in range(NDC):
                                lhsT = w_sb[:, dc * FPC + ft * 128:
                                            dc * FPC + ft * 128 + 128]
                                nc.tensor.matmul(
                                    psq[:], lhsT,
                                    xt_sb[:, dc * S + sc * 512:
                                          dc * S + sc * 512 + 512],
                                    start=(dc == 0), stop=(dc == NDC - 1))
                            bias_ap = bqk_sb[:, 2 * wi + ft: 2 * wi + ft + 1]
                            dslc = dstt[ft][:, sc * 512:(sc + 1) * 512]
                            if wi == 0:
                                nc.scalar.activation(dslc, psq[:], Ident,
                                                     bias=bias_ap)
                            else:
                                nc.vector.tensor_scalar_add(dslc, psq[:],
                                                            bias_ap)

                    # V natural + fused bias for these 4 S-tiles
                    for st in range(4 * sc, 4 * sc + 4):
                        vp = vpps.tile([128, FPC], F32, tag="vp",
                                       name=f"vp{st}")
                        for dc in range(NDC):
                            nc.tensor.matmul(
                                vp[:],
                                xt_sb[:, dc * S + st * 128:
                                      dc * S + st * 128 + 128],
                                wv_sb[:, dc * FPC:(dc + 1) * FPC],
                                start=(dc == 0), stop=False)
                        nc.tensor.matmul(vp[:], on_sb[:], bv_sb[:],
                                         start=False, stop=True)
                        nc.scalar.copy(v_sb[:, st * FPC:(st + 1) * FPC],
                                       vp[:])
                        srcv = vp[:].rearrange("p (hl c) -> p hl c", hl=HPC)
                        dst3 = vb_sb[:].rearrange(
                            "p (hl t) -> p hl t", hl=HPC)[
                            :, :, st * 128: st * 128 + HD]
                        if st % 2 == 0:
                            nc.vector.tensor_copy(dst3, srcv)
                        else:
                            nc.scalar.copy(dst3, srcv)

            # pools that reuse the space freed by xt: wp, K-natural
            # collector, attention transients, out staging
            with tc.tile_pool(name="late", bufs=1) as late, \
                 tc.tile_pool(name="pt", bufs=4) as ptp, \
                 tc.tile_pool(name="rs", bufs=4) as rsp, \
                 tc.tile_pool(name="osb", bufs=2) as osbp:

                wp_sb = late.tile([128, 2 * D], F32R, tag="wp")
                nc.scalar.dma_start(out=wp_sb[:], in_=wp[:])
                kn_sb = late.tile([128, 2 * NQT * 128], F32, tag="kn")

                # ---- stage 3: K natural + present outputs ----
                with tc.tile_pool(name="kn_ps", bufs=3, space="PSUM") as knps:
                    for ft in range(2):
                        for st in range(NQT):
                            kp = knps.tile([128, 128], F32R, tag="knp")
                            nc.tensor.transpose(
                                kp[:], kt_sb[ft][:, st * 128:(st + 1) * 128],
                                idr_sb[:])
                            dstk = kn_sb[:, ft * S + st * 128:
                                         ft * S + st * 128 + 128]
                            if st % 2 == 0:
                                nc.scalar.copy(dstk, kp[:].bitcast(F32))
                            else:
                                nc.vector.tensor_copy(dstk, kp[:].bitcast(F32))
                    # one DMA per head
                    kn4 = kn_sb[:].rearrange(
                        "p (ft st h d) -> p ft st h d", ft=2, st=NQT, h=2)
                    for ft in range(2):
                        for h2 in range(2):
                            nc.sync.dma_start(
                                out=pk[2 * ft + h2, :, :].rearrange(
                                    "(st p) d -> p st d", p=128),
                                in_=kn4[:, ft, :, h2, :])
                    v4 = v_sb[:].rearrange(
                        "p (st hl d) -> p st hl d", st=NQT, hl=HPC)
                    for hl in range(HPC):
                        nc.sync.dma_start(
                            out=pv[hl, :, :].rearrange(
                                "(st p) d -> p st d", p=128),
                            in_=v4[:, :, hl, :])

                # ---- stage 4: attention per head (software-pipelined).
                # The last head also interleaves the out-projection so PE
                # fills exp-latency stalls with useful work. ----
                def attention_head(hl, stps, ops, norm_lag, per_chunk_hook):
                    ft, po = hl // 2, (hl % 2) * 64
                    kth = kt_sb[ft]
                    qth = qt_sb[ft]
                    oth = ot_sb[ft]
                    vbh = vb_sb[:, hl * NQT * 128:(hl + 1) * NQT * 128]

                    pts = [None] * NQT
                    osums = [None] * NQT

                    def qk_exp_mask(c):
                        qw = min(384, S - c * 128)
                        sp = stps.tile([128, 384], F32, tag="sp",
                                       name=f"sp{hl}_{c}")
                        nc.tensor.matmul(
                            sp[:, :qw],
                            kth[po:po + 64, c * 128:(c + 1) * 128],
                            qth[po:po + 64, c * 128:c * 128 + qw],
                            start=True, stop=False)
                        # additive mask (-1e30 off-window) via I.T @ madd
                        nc.tensor.matmul(
                            sp[:, :qw], idr_sb[:], madd_sb[:, :qw],
                            start=False, stop=True)
                        pt = ptp.tile([128, 384], F16, tag="pt",
                                      name=f"pt{hl}_{c}")
                        nc.scalar.activation(pt[:, :qw], sp[:, :qw], Exp)
                        pts[c] = pt

                    def osum(c):
                        if osums[c] is None:
                            osums[c] = ops.tile([128, 128], F32, tag="o",
                                                name=f"o{hl}_{c}")
                        return osums[c]

                    def normalize(c):
                        o_cur = osums[c]
                        rec = rsp.tile([64, 128], F32, tag="rec",
                                       name=f"rec{hl}_{c}")
                        nc.vector.reciprocal(rec[:], o_cur[64:128, :])
                        nc.vector.tensor_mul(
                            oth[po:po + 64, c * 128:(c + 1) * 128],
                            o_cur[0:64, :], rec[:])
                        osums[c] = None

                    qk_exp_mask(0)
                    qk_exp_mask(1)
                    for c in range(NQT):
                        if c + 2 < NQT:
                            qk_exp_mask(c + 2)
                        pt = pts[c]
                        qw = min(384, S - c * 128)
                        lhsT = vbh[:, c * 128:(c + 1) * 128]
                        nc.tensor.matmul(osum(c)[:], lhsT, pt[:, 0:128],
                                         start=(c == 0), stop=True)
                        if c + 1 < NQT:
                            nc.tensor.matmul(osum(c + 1)[:], lhsT,
                                             pt[:, 128:256],
                                             start=(c == 0), stop=False)
                        if c + 2 < NQT and qw > 256:
                            nc.tensor.matmul(osum(c + 2)[:], lhsT,
                                             pt[:, 256:384],
                                             start=True, stop=False)
                        # normalization lags the AV matmuls so the DVE
                        # stream never round-trips against PE
                        if c >= norm_lag:
                            normalize(c - norm_lag)
                            if per_chunk_hook is not None:
                                per_chunk_hook(c - norm_lag)
                        pts[c] = None
                    for c in range(NQT - norm_lag, NQT):
                        normalize(c)
                        if per_chunk_hook is not None:
                            per_chunk_hook(c)

                osb_state = {}

                def outproj_tile(st, opps):
                    # called once per q-tile st (in order) after all heads
                    # normalized it
                    st2, i = st // 2, st % 2
                    if i == 0:
                        osb_state["t"] = osbp.tile([128, 2 * D], F32,
                                                   tag="osb",
                                                   name=f"osb{st2}")
                    o_t = osb_state["t"]
                    for half in range(2):
                        op = opps.tile([128, 512], F32, tag="op",
                                       name=f"op{st}_{half}")
                        nc.tensor.matmul(
                            op[:], ot_sb[0][:, st * 128:(st + 1) * 128],
                            wp_sb[:, half * 512: half * 512 + 512],
                            start=True, stop=False)
                        nc.tensor.matmul(
                            op[:], ot_sb[1][:, st * 128:(st + 1) * 128],
                            wp_sb[:, D + half * 512: D + half * 512 + 512],
                            start=False, stop=True)
                        dsl = o_t[:, i * D + half * 512:
                                  i * D + (half + 1) * 512]
                        if half == 0:
                            nc.scalar.copy(dsl, op[:])
                        else:
                            nc.vector.tensor_copy(dsl, op[:])
                    if i == 1:
                        nc.sync.dma_start(
                            out=outp[st2 * 256:(st2 + 1) * 256, :].rearrange(
                                "(j p) d -> p j d", p=128),
                            in_=o_t[:].rearrange("p (j d) -> p j d", j=2))

                with tc.tile_pool(name="st_ps", bufs=3, space="PSUM") as stps, \
                     tc.tile_pool(name="o_ps", bufs=5, space="PSUM") as ops:
                    for hl in range(HPC - 1):
                        attention_head(hl, stps, ops, 2, None)

                with tc.tile_pool(name="st_ps2", bufs=2, space="PSUM") as stps, \
                     tc.tile_pool(name="o_ps2", bufs=4, space="PSUM") as ops, \
                     tc.tile_pool(name="op_ps", bufs=2, space="PSUM") as opps:
                    attention_head(HPC - 1, stps, ops, 1,
                                   lambda st: outproj_tile(st, opps))

    nc.compile()
    return nc


def _prep_in_maps(x, w_attn, b_attn, w_proj):
    """Per-core input dicts (host-side sharding + layout prep)."""
    x = np.ascontiguousarray(np.asarray(x, dtype=np.float32))
    w_attn = np.asarray(w_attn, dtype=np.float32)
    b_attn = np.asarray(b_attn, dtype=np.float32)
    w_proj = np.asarray(w_proj, dtype=np.float32)

    ident = np.eye(128, dtype=np.float32)
    ql = np.arange(128)[None, :]
    kl = np.arange(128)[:, None]
    neg = np.float32(-1e30)
    madd = np.concatenate(
        [np.where(ql >= kl, np.float32(0), neg),
         np.zeros((128, 128), np.float32),
         np.where(ql < kl, np.float32(0), neg)], axis=1).astype(np.float32)
    onesr = np.ones((1, 128), dtype=np.float32)

    def chunk_w(w_cols):  # [D, FPC] -> [128, NDC*FPC]
        return w_cols.reshape(NDC, 128, FPC).transpose(1, 0, 2).reshape(
            128, NDC * FPC)

    in_maps = []
    for core in range(NCORES):
        b, hg = core // 4, core % 4
        cols = slice(hg * FPC, (hg + 1) * FPC)
        kcols = slice(D + hg * FPC, D + (hg + 1) * FPC)
        vcols = slice(2 * D + hg * FPC, 2 * D + (hg + 1) * FPC)
        rows = slice(hg * FPC, (hg + 1) * FPC)
        wqkv = np.concatenate(
            [chunk_w(w_attn[:, cols] * np.float32(SCALE)),
             chunk_w(w_attn[:, kcols]),
             chunk_w(w_attn[:, vcols])], axis=1)
        bqk = np.stack(
            [(b_attn[cols] * np.float32(SCALE)).reshape(2, 128)[0],
             (b_attn[cols] * np.float32(SCALE)).reshape(2, 128)[1],
             b_attn[kcols].reshape(2, 128)[0],
             b_attn[kcols].reshape(2, 128)[1]], axis=1)
        in_maps.append({
            "xb": x[b],
            "wqkv": np.ascontiguousarray(wqkv),
            "wp": np.ascontiguousarray(
                w_proj[rows, :].reshape(2, 128, D).transpose(1, 0, 2).reshape(
                    128, 2 * D)),
            "bqk": np.ascontiguousarray(bqk),
            "bv": b_attn[vcols].reshape(1, FPC).copy(),
            "ident_r": ident,
            "madd": madd,
            "onesr": onesr,
        })
    return in_maps


def kernel(x, w_attn, b_attn, w_proj, b_proj):
    if "nc" not in _CACHE:
        _CACHE["nc"] = _build_program()
    nc = _CACHE["nc"]

    in_maps = _prep_in_maps(x, w_attn, b_attn, w_proj)
    res = run_bass_kernel_spmd(nc, in_maps, core_ids=list(range(NCORES)))

    b_proj = np.asarray(b_proj, dtype=np.float32)
    out = np.zeros((B, S, D), dtype=np.float32)
    present = np.zeros((B, 2, N_HEAD, S, HD), dtype=np.float32)
    for core in range(NCORES):
        b, hg = core // 4, core % 4
        r = res.results[core]
        out[b] += r["outp"]
        present[b, 0, hg * HPC:(hg + 1) * HPC] = r["pk"]
        present[b, 1, hg * HPC:(hg + 1) * HPC] = r["pv"]
    out += b_proj
    return out, present


# revision 15
# speedup vs baseline: 1.1782x; 1.0338x over previous
"""Trainium2 Bass kernel for sliding-window causal attention block.

Reference computation (B=2, S=2048, D=1024, H=16, hd=64, WINDOW=256):
    c = x @ w_attn + b_attn ; q,k,v = split(c)
    present = stack([k, v]) as [B,2,H,S,hd]
    att = softmax(mask(q k^T / sqrt(hd))) @ v
    out = att @ w_proj + b_proj

Sharding: 8 cores = 2 batches x 4 head-groups (4 heads each).
Per core: QKV projection for its 256 q/k/v features (column-sharded
w_attn), attention for its 4 heads, and a partial out-projection
(row-sharded w_proj). Host sums the 4 partials per batch and adds
b_proj exactly.

Layout strategy on-core:
  x[b] is PE-transposed once to xT [D, S] (f32r, 4 transposes batched
  per PSUM bank, eviction split across DVE/ACT).
  Q^T, K^T produced feature-major [feat, S] (f32r), V natural [S, feat].
  Scores are computed directly transposed: S^T[kpos, q] tile per
  128-kpos chunk covering its 384 valid q columns (window 256 spans 3
  q-tiles), so softmax needs no P transposes. Masks: causal triangle on
  the left third, all-valid middle, anti-causal triangle right.
  exp on ACT -> P^T fp16; AV matmul uses V|ones fp16 where columns
  64:128 are all ones, so the softmax denominator lands replicated on
  PSUM partitions 64:128 -> 64-lane reciprocal + multiply on DVE.
  O^T f32r; out-projection back to natural [S, D] layout.
  DMAs are batched aggressively (the SP sequencer pays ~0.6us per DMA
  instruction): 8 input DMAs for x, 1 for all of wq/wk/wv, single
  per-head DMAs for the present k/v outputs, 2-S-tile DMAs for out.
"""

import sys

sys.path.insert(0, "/opt/trn_rl_repo")

import numpy as np

import concourse.bass as bass  # noqa: F401  (bass must import before bacc)
import concourse.mybir as mybir
from concourse import bacc
from concourse.tile import TileContext
from concourse.bass_utils import run_bass_kernel_spmd

F32 = mybir.dt.float32
F32R = mybir.dt.float32r
F16 = mybir.dt.float16

B, S, D = 2, 2048, 1024
N_HEAD = 16
HD = 64
WINDOW = 256
NCORES = 8
HPC = N_HEAD // 4  # heads per core = 4
FPC = HPC * HD  # features per core = 256
NQT = S // 128  # 16 q/kpos tiles
NDC = D // 128  # 8 contraction chunks
SCALE = 1.0 / np.sqrt(HD)

_CACHE = {}


def _build_program():
    nc = bacc.Bacc("TRN2", target_bir_lowering=False, debug=False,
                   num_devices=NCORES)

    # ---- DRAM I/O ----
    xb = nc.dram_tensor("xb", [S, D], F32R, kind="ExternalInput")
    # wq | wk | wv, each pre-chunked to [128, NDC*FPC]
    wqkv = nc.dram_tensor("wqkv", [128, 3 * NDC * FPC], F32R,
                          kind="ExternalInput")
    wp = nc.dram_tensor("wp", [128, 2 * D], F32R, kind="ExternalInput")
    bqk = nc.dram_tensor("bqk", [128, 4], F32, kind="ExternalInput")
    bv = nc.dram_tensor("bv", [1, FPC], F32R, kind="ExternalInput")
    ident_r = nc.dram_tensor("ident_r", [128, 128], F32R, kind="ExternalInput")
    madd = nc.dram_tensor("madd", [128, 384], F32R, kind="ExternalInput")
    onesr = nc.dram_tensor("onesr", [1, 128], F32R, kind="ExternalInput")

    outp = nc.dram_tensor("outp", [S, D], F32, kind="ExternalOutput")
    pk = nc.dram_tensor("pk", [HPC, S, HD], F32, kind="ExternalOutput")
    pv = nc.dram_tensor("pv", [HPC, S, HD], F32, kind="ExternalOutput")

    Exp = mybir.ActivationFunctionType.Exp
    Ident = mybir.ActivationFunctionType.Identity

    with TileContext(nc) as tc:
        with tc.tile_pool(name="const", bufs=1) as cpool, \
             tc.tile_pool(name="qkv", bufs=1) as qkv, \
             tc.tile_pool(name="ot", bufs=1) as otp:

            # identity first (x transposes depend on it)
            idr_sb = cpool.tile([128, 128], F32R, tag="idr")
            nc.sync.dma_start(out=idr_sb[:], in_=ident_r[:])

            # persistent activations
            qt_sb = [qkv.tile([128, S], F32R, tag=f"qt{i}", name=f"qt{i}")
                     for i in range(2)]
            kt_sb = [qkv.tile([128, S], F32R, tag=f"kt{i}", name=f"kt{i}")
                     for i in range(2)]
            v_sb = qkv.tile([128, NQT * FPC], F32, tag="v")
            # V|ones fp16 per head: head hl block [128, NQT*128];
            # cols 64:128 of each chunk stay 1.0 (denominator trick)
            vb_sb = qkv.tile([128, HPC * NQT * 128], F16, tag="vb")
            kn_sb = qkv.tile([128, 2 * NQT * 128], F32, tag="kn")
            ot_sb = [otp.tile([128, S], F32R, tag=f"ot{i}", name=f"ot{i}")
                     for i in range(2)]

            nc.gpsimd.memset(vb_sb[:], 1.0)

            # ---- stage 1+2: x load/transpose + QKV projection ----
            with tc.tile_pool(name="xt", bufs=1) as xtp, \
                 tc.tile_pool(name="xload", bufs=3) as xlp, \
                 tc.tile_pool(name="tp_ps", bufs=2, space="PSUM") as tpps, \
                 tc.tile_pool(name="qk_ps", bufs=4, space="PSUM") as qkps, \
                 tc.tile_pool(name="vp_ps", bufs=2, space="PSUM") as vpps:

                xt_sb = xtp.tile([128, NDC * S], F32R, tag="xt")
                xt3 = xt_sb[:].rearrange("p (dc s) -> p dc s", dc=NDC)

                # prefetch first x tiles so PE transposes start immediately
                x_pre = {}
                for st in range(3):
                    x_t = xlp.tile([128, D], F32R, tag="x", name=f"x{st}")
                    nc.sync.dma_start(out=x_t[:],
                                      in_=xb[st * 128:(st + 1) * 128, :])
                    x_pre[st] = x_t

                # weights next in emission (three 1MB transfers so later x
                # tiles interleave on the DMA engines)
                wqkv_sb = cpool.tile([128, 3 * NDC * FPC], F32R, tag="wqkv")
                NW = NDC * FPC
                for wi3 in range(3):
                    nc.scalar.dma_start(
                        out=wqkv_sb[:, wi3 * NW:(wi3 + 1) * NW],
                        in_=wqkv[:, wi3 * NW:(wi3 + 1) * NW])
                wq_sb = wqkv_sb[:, 0:NDC * FPC]
                wk_sb = wqkv_sb[:, NDC * FPC:2 * NDC * FPC]
                wv_sb = wqkv_sb[:, 2 * NDC * FPC:3 * NDC * FPC]
                bqk_sb = cpool.tile([128, 4], F32, tag="bqk")
                nc.scalar.dma_start(out=bqk_sb[:], in_=bqk[:])
                bv_sb = cpool.tile([1, FPC], F32R, tag="bv")
                nc.scalar.dma_start(out=bv_sb[:], in_=bv[:])
                madd_sb = cpool.tile([128, 384], F32R, tag="madd")
                nc.scalar.dma_start(out=madd_sb[:], in_=madd[:])
                on_sb = cpool.tile([1, 128], F32R, tag="on")
                nc.scalar.dma_start(out=on_sb[:], in_=onesr[:])

                # interleave x load/transpose with QKV so PE never waits
                # for the full x transfer (PE streams are in-order)
                for sc in range(4):
                    for st in range(4 * sc, 4 * sc + 4):
                        if st in x_pre:
                            x_t = x_pre.pop(st)
                        else:
                            x_t = xlp.tile([128, D], F32R, tag="x",
                                           name=f"x{st}")
                            nc.sync.dma_start(
                                out=x_t[:],
                                in_=xb[st * 128:(st + 1) * 128, :])
                        for dg in range(2):
                            ps = tpps.tile([128, 512], F32R, tag="tp",
                                           name=f"tp{st}_{dg}")
                            for j in range(4):
                                dc = dg * 4 + j
                                nc.tensor.transpose(
                                    ps[:, j * 128:(j + 1) * 128],
                                    x_t[:, dc * 128:(dc + 1) * 128],
                                    idr_sb[:])
                            dst = xt3[:, dg * 4:(dg + 1) * 4,
                                      st * 128:st * 128 + 128]
                            srcp = ps[:].rearrange("p (j s) -> p j s", j=4)
                            if (st + dg) % 2 == 0:
                                nc.vector.tensor_copy(dst, srcp)
                            else:
                                nc.scalar.copy(dst, srcp)

                    # Q^T / K^T columns for this 512-wide S group.
                    # Q evicts on ACT (+bias), K evicts on DVE (+bias).
                    for wi, (w_sb, dstt) in enumerate(((wq_sb, qt_sb),
                                                       (wk_sb, kt_sb))):
                        for ft in range(2):
                            psq = qkps.tile([128, 512], F32, tag="qk",
                                            name=f"qkps{sc}_{wi}{ft}")
                            for dc in range(NDC):
                                lhsT = w_sb[:, dc * FPC + ft * 128:
                                            dc * FPC + ft * 128 + 128]
                                nc.tensor.matmul(
                                    psq[:], lhsT,
                                    xt_sb[:, dc * S + sc * 512:
                                          dc * S + sc * 512 + 512],
                                    start=(dc == 0), stop=(dc == NDC - 1))
                            bias_ap = bqk_sb[:, 2 * wi + ft: 2 * wi + ft + 1]
                            dslc = dstt[ft][:, sc * 512:(sc + 1) * 512]
                            if wi == 0:
                                nc.scalar.activation(dslc, psq[:], Ident,
                                                     bias=bias_ap)
                            else:
                                nc.vector.tensor_scalar_add(dslc, psq[:],
                                                            bias_ap)

                    # K natural for `present`: 4 batched PE transposes
                    # per ft into one PSUM bank, single ACT evict
                    for ft in range(2):
                        psn = tpps.tile([128, 512], F32R, tag="tp",
                                        name=f"kn{sc}_{ft}")
                        for j in range(4):
                            st = 4 * sc + j
                            nc.tensor.transpose(
                                psn[:, j * 128:(j + 1) * 128],
                                kt_sb[ft][:, st * 128:(st + 1) * 128],
                                idr_sb[:])
                        nc.scalar.copy(
                            kn_sb[:, ft * S + sc * 512:
                                  ft * S + (sc + 1) * 512],
                            psn[:].bitcast(F32))

                    # V natural + fused bias for these 4 S-tiles
                    for st in range(4 * sc, 4 * sc + 4):
                        vp = vpps.tile([128, FPC], F32, tag="vp",
                                       name=f"vp{st}")
                        for dc in range(NDC):
                            nc.tensor.matmul(
                                vp[:],
                                xt_sb[:, dc * S + st * 128:
                                      dc * S + st * 128 + 128],
                                wv_sb[:, dc * FPC:(dc + 1) * FPC],
                                start=(dc == 0), stop=False)
                        nc.tensor.matmul(vp[:], on_sb[:], bv_sb[:],
                                         start=False, stop=True)
                        nc.scalar.copy(v_sb[:, st * FPC:(st + 1) * FPC],
                                       vp[:])
                        srcv = vp[:].rearrange("p (hl c) -> p hl c", hl=HPC)
                        dst3 = vb_sb[:].rearrange(
                            "p (hl t) -> p hl t", hl=HPC)[
                            :, :, st * 128: st * 128 + HD]
                        if st % 2 == 0:
                            nc.vector.tensor_copy(dst3, srcv)
                        else:
                            nc.scalar.copy(dst3, srcv)

            # pools that reuse the space freed by xt: wp, K-natural
            # collector, attention transients, out staging
            with tc.tile_pool(name="late", bufs=1) as late, \
                 tc.tile_pool(name="pt", bufs=4) as ptp, \
                 tc.tile_pool(name="rs", bufs=4) as rsp, \
                 tc.tile_pool(name="osb", bufs=2) as osbp:

                wp_sb = late.tile([128, 2 * D], F32R, tag="wp")
                nc.scalar.dma_start(out=wp_sb[:], in_=wp[:])

                # ---- stage 3: present outputs (k/v computed above) ----
                kn4 = kn_sb[:].rearrange(
                    "p (ft st h d) -> p ft st h d", ft=2, st=NQT, h=2)
                for ft in range(2):
                    for h2 in range(2):
                        nc.sync.dma_start(
                            out=pk[2 * ft + h2, :, :].rearrange(
                                "(st p) d -> p st d", p=128),
                            in_=kn4[:, ft, :, h2, :])
                v4 = v_sb[:].rearrange(
                    "p (st hl d) -> p st hl d", st=NQT, hl=HPC)
                for hl in range(HPC):
                    nc.sync.dma_start(
                        out=pv[hl, :, :].rearrange(
                            "(st p) d -> p st d", p=128),
                        in_=v4[:, :, hl, :])

                # ---- stage 4: attention per head (software-pipelined).
                # The last head also interleaves the out-projection so PE
                # fills exp-latency stalls with useful work. ----
                def attention_head(hl, stps, ops, norm_lag, per_chunk_hook):
                    ft, po = hl // 2, (hl % 2) * 64
                    kth = kt_sb[ft]
                    qth = qt_sb[ft]
                    oth = ot_sb[ft]
                    vbh = vb_sb[:, hl * NQT * 128:(hl + 1) * NQT * 128]

                    pts = [None] * NQT
                    osums = [None] * NQT

                    def qk_exp_mask(c):
                        qw = min(384, S - c * 128)
                        sp = stps.tile([128, 384], F32, tag="sp",
                                       name=f"sp{hl}_{c}")
                        nc.tensor.matmul(
                            sp[:, :qw],
                            kth[po:po + 64, c * 128:(c + 1) * 128],
                            qth[po:po + 64, c * 128:c * 128 + qw],
                            start=True, stop=False)
                        # additive mask (-1e30 off-window) via I.T @ madd
                        nc.tensor.matmul(
                            sp[:, :qw], idr_sb[:], madd_sb[:, :qw],
                            start=False, stop=True)
                        pt = ptp.tile([128, 384], F16, tag="pt",
                                      name=f"pt{hl}_{c}")
                        nc.scalar.activation(pt[:, :qw], sp[:, :qw], Exp)
                        pts[c] = pt

                    def osum(c):
                        if osums[c] is None:
                            osums[c] = ops.tile([128, 128], F32, tag="o",
                                                name=f"o{hl}_{c}")
                        return osums[c]

                    def normalize(c):
                        o_cur = osums[c]
                        rec = rsp.tile([64, 128], F32, tag="rec",
                                       name=f"rec{hl}_{c}")
                        nc.vector.reciprocal(rec[:], o_cur[64:128, :])
                        nc.vector.tensor_mul(
                            oth[po:po + 64, c * 128:(c + 1) * 128],
                            o_cur[0:64, :], rec[:])
                        osums[c] = None

                    qk_exp_mask(0)
                    qk_exp_mask(1)
                    for c in range(NQT):
                        if c + 2 < NQT:
                            qk_exp_mask(c + 2)
                        pt = pts[c]
                        qw = min(384, S - c * 128)
                        lhsT = vbh[:, c * 128:(c + 1) * 128]
                        nc.tensor.matmul(osum(c)[:], lhsT, pt[:, 0:128],
                                         start=(c == 0), stop=True)
                        if c + 1 < NQT:
                            nc.tensor.matmul(osum(c + 1)[:], lhsT,
                                             pt[:, 128:256],
                                             start=(c == 0), stop=False)
                        if c + 2 < NQT and qw > 256:
                            nc.tensor.matmul(osum(c + 2)[:], lhsT,
                                             pt[:, 256:384],
                                             start=True, stop=False)
                        # normalization lags the AV matmuls so the DVE
                        # stream never round-trips against PE
                        if c >= norm_lag:
                            normalize(c - norm_lag)
                            if per_chunk_hook is not None:
                                per_chunk_hook(c - norm_lag)
                        pts[c] = None
                    for c in range(NQT - norm_lag, NQT):
                        normalize(c)
                        if per_chunk_hook is not None:
                            per_chunk_hook(c)

                osb_state = {}

                def outproj_tile(st, opps):
                    # called once per q-tile st (in order) after all heads
                    # normalized it
                    st2, i = st // 2, st % 2
                    if i == 0:
                        osb_state["t"] = osbp.tile([128, 2 * D], F32,
                                                   tag="osb",
                                                   name=f"osb{st2}")
                    o_t = osb_state["t"]
                    for half in range(2):
                        op = opps.tile([128, 512], F32, tag="op",
                                       name=f"op{st}_{half}")
                        nc.tensor.matmul(
                            op[:], ot_sb[0][:, st * 128:(st + 1) * 128],
                            wp_sb[:, half * 512: half * 512 + 512],
                            start=True, stop=False)
                        nc.tensor.matmul(
                            op[:], ot_sb[1][:, st * 128:(st + 1) * 128],
                            wp_sb[:, D + half * 512: D + half * 512 + 512],
                            start=False, stop=True)
                        dsl = o_t[:, i * D + half * 512:
                                  i * D + (half + 1) * 512]
                        if half == 0:
                            nc.scalar.copy(dsl, op[:])
                        else:
                            nc.vector.tensor_copy(dsl, op[:])
                    if i == 1:
                        nc.sync.dma_start(
                            out=outp[st2 * 256:(st2 + 1) * 256, :].rearrange(
                                "(j p) d -> p j d", p=128),
                            in_=o_t[:].rearrange("p (j d) -> p j d", j=2))

                with tc.tile_pool(name="st_ps", bufs=3, space="PSUM") as stps, \
                     tc.tile_pool(name="o_ps", bufs=5, space="PSUM") as ops:
                    for hl in range(HPC - 1):
                        attention_head(hl, stps, ops, 2, None)

                with tc.tile_pool(name="st_ps2", bufs=2, space="PSUM") as stps, \
                     tc.tile_pool(name="o_ps2", bufs=4, space="PSUM") as ops, \
                     tc.tile_pool(name="op_ps", bufs=2, space="PSUM") as opps:
                    attention_head(HPC - 1, stps, ops, 1,
                                   lambda st: outproj_tile(st, opps))

    nc.compile()
    return nc


def _prep_in_maps(x, w_attn, b_attn, w_proj):
    """Per-core input dicts (host-side sharding + layout prep)."""
    x = np.ascontiguousarray(np.asarray(x, dtype=np.float32))
    w_attn = np.asarray(w_attn, dtype=np.float32)
    b_attn = np.asarray(b_attn, dtype=np.float32)
    w_proj = np.asarray(w_proj, dtype=np.float32)

    ident = np.eye(128, dtype=np.float32)
    ql = np.arange(128)[None, :]
    kl = np.arange(128)[:, None]
    neg = np.float32(-1e30)
    madd = np.concatenate(
        [np.where(ql >= kl, np.float32(0), neg),
         np.zeros((128, 128), np.float32),
         np.where(ql < kl, np.float32(0), neg)], axis=1).astype(np.float32)
    onesr = np.ones((1, 128), dtype=np.float32)

    def chunk_w(w_cols):  # [D, FPC] -> [128, NDC*FPC]
        return w_cols.reshape(NDC, 128, FPC).transpose(1, 0, 2).reshape(
            128, NDC * FPC)

    in_maps = []
    for core in range(NCORES):
        b, hg = core // 4, core % 4
        cols = slice(hg * FPC, (hg + 1) * FPC)
        kcols = slice(D + hg * FPC, D + (hg + 1) * FPC)
        vcols = slice(2 * D + hg * FPC, 2 * D + (hg + 1) * FPC)
        rows = slice(hg * FPC, (hg + 1) * FPC)
        wqkv = np.concatenate(
            [chunk_w(w_attn[:, cols] * np.float32(SCALE)),
             chunk_w(w_attn[:, kcols]),
             chunk_w(w_attn[:, vcols])], axis=1)
        bqk = np.stack(
            [(b_attn[cols] * np.float32(SCALE)).reshape(2, 128)[0],
             (b_attn[cols] * np.float32(SCALE)).reshape(2, 128)[1],
             b_attn[kcols].reshape(2, 128)[0],
             b_attn[kcols].reshape(2, 128)[1]], axis=1)
        in_maps.append({
            "xb": x[b],
            "wqkv": np.ascontiguousarray(wqkv),
            "wp": np.ascontiguousarray(
                w_proj[rows, :].reshape(2, 128, D).transpose(1, 0, 2).reshape(
                    128, 2 * D)),
            "bqk": np.ascontiguousarray(bqk),
            "bv": b_attn[vcols].reshape(1, FPC).copy(),
            "ident_r": ident,
            "madd": madd,
            "onesr": onesr,
        })
    return in_maps


def kernel(x, w_attn, b_attn, w_proj, b_proj):
    if "nc" not in _CACHE:
        _CACHE["nc"] = _build_program()
    nc = _CACHE["nc"]

    in_maps = _prep_in_maps(x, w_attn, b_attn, w_proj)
    res = run_bass_kernel_spmd(nc, in_maps, core_ids=list(range(NCORES)))

    b_proj = np.asarray(b_proj, dtype=np.float32)
    out = np.zeros((B, S, D), dtype=np.float32)
    present = np.zeros((B, 2, N_HEAD, S, HD), dtype=np.float32)
    for core in range(NCORES):
        b, hg = core // 4, core % 4
        r = res.results[core]
        out[b] += r["outp"]
        present[b, 0, hg * HPC:(hg + 1) * HPC] = r["pk"]
        present[b, 1, hg * HPC:(hg + 1) * HPC] = r["pv"]
    out += b_proj
    return out, present


# revision 17
# speedup vs baseline: 1.1786x; 1.0004x over previous
"""Trainium2 Bass kernel for sliding-window causal attention block.

Reference computation (B=2, S=2048, D=1024, H=16, hd=64, WINDOW=256):
    c = x @ w_attn + b_attn ; q,k,v = split(c)
    present = stack([k, v]) as [B,2,H,S,hd]
    att = softmax(mask(q k^T / sqrt(hd))) @ v
    out = att @ w_proj + b_proj

Sharding: 8 cores = 2 batches x 4 head-groups (4 heads each).
Per core: QKV projection for its 256 q/k/v features (column-sharded
w_attn), attention for its 4 heads, and a partial out-projection
(row-sharded w_proj). Host sums the 4 partials per batch and adds
b_proj exactly.

Layout strategy on-core:
  x[b] is PE-transposed once to xT [D, S] (f32r, 4 transposes batched
  per PSUM bank, eviction split across DVE/ACT).
  Q^T, K^T produced feature-major [feat, S] (f32r), V natural [S, feat].
  Scores are computed directly transposed: S^T[kpos, q] tile per
  128-kpos chunk covering its 384 valid q columns (window 256 spans 3
  q-tiles), so softmax needs no P transposes. Masks: causal triangle on
  the left third, all-valid middle, anti-causal triangle right.
  exp on ACT -> P^T fp16; AV matmul uses V|ones fp16 where columns
  64:128 are all ones, so the softmax denominator lands replicated on
  PSUM partitions 64:128 -> 64-lane reciprocal + multiply on DVE.
  O^T f32r; out-projection back to natural [S, D] layout.
  DMAs are batched aggressively (the SP sequencer pays ~0.6us per DMA
  instruction): 8 input DMAs for x, 1 for all of wq/wk/wv, single
  per-head DMAs for the present k/v outputs, 2-S-tile DMAs for out.
"""

import sys

sys.path.insert(0, "/opt/trn_rl_repo")

import numpy as np

import concourse.bass as bass  # noqa: F401  (bass must import before bacc)
import concourse.mybir as mybir
from concourse import bacc
from concourse.tile import TileContext
from concourse.bass_utils import run_bass_kernel_spmd

F32 = mybir.dt.float32
F32R = mybir.dt.float32r
F16 = mybir.dt.float16

B, S, D = 2, 2048, 1024
N_HEAD = 16
HD = 64
WINDOW = 256
NCORES = 8
HPC = N_HEAD // 4  # heads per core = 4
FPC = HPC * HD  # features per core = 256
NQT = S // 128  # 16 q/kpos tiles
NDC = D // 128  # 8 contraction chunks
SCALE = 1.0 / np.sqrt(HD)

_CACHE = {}


def _build_program():
    nc = bacc.Bacc("TRN2", target_bir_lowering=False, debug=False,
                   num_devices=NCORES)

    # ---- DRAM I/O ----
    xb = nc.dram_tensor("xb", [S, D], F32R, kind="ExternalInput")
    # wq | wk | wv, each pre-chunked to [128, NDC*FPC]
    wqkv = nc.dram_tensor("wqkv", [128, 3 * NDC * FPC], F32R,
                          kind="ExternalInput")
    wp = nc.dram_tensor("wp", [128, 2 * D], F32R, kind="ExternalInput")
    bqk = nc.dram_tensor("bqk", [128, 4], F32, kind="ExternalInput")
    bv = nc.dram_tensor("bv", [1, FPC], F32R, kind="ExternalInput")
    ident_r = nc.dram_tensor("ident_r", [128, 128], F32R, kind="ExternalInput")
    madd = nc.dram_tensor("madd", [128, 256], F16, kind="ExternalInput")
    idh = nc.dram_tensor("idh", [128, 128], F16, kind="ExternalInput")
    onesr = nc.dram_tensor("onesr", [1, 128], F32R, kind="ExternalInput")

    outp = nc.dram_tensor("outp", [S, D], F32, kind="ExternalOutput")
    pk = nc.dram_tensor("pk", [HPC, S, HD], F32, kind="ExternalOutput")
    pv = nc.dram_tensor("pv", [HPC, S, HD], F32, kind="ExternalOutput")

    Exp = mybir.ActivationFunctionType.Exp
    Ident = mybir.ActivationFunctionType.Identity

    with TileContext(nc) as tc:
        with tc.tile_pool(name="const", bufs=1) as cpool, \
             tc.tile_pool(name="qkv", bufs=1) as qkv, \
             tc.tile_pool(name="ot", bufs=1) as otp:

            # identity first (x transposes depend on it)
            idr_sb = cpool.tile([128, 128], F32R, tag="idr")
            nc.sync.dma_start(out=idr_sb[:], in_=ident_r[:])

            # persistent activations
            qt_sb = [qkv.tile([128, S], F32R, tag=f"qt{i}", name=f"qt{i}")
                     for i in range(2)]
            kt_sb = [qkv.tile([128, S], F32R, tag=f"kt{i}", name=f"kt{i}")
                     for i in range(2)]
            v_sb = qkv.tile([128, NQT * FPC], F32, tag="v")
            # V|ones fp16 per head: head hl block [128, NQT*128];
            # cols 64:128 of each chunk stay 1.0 (denominator trick)
            vb_sb = qkv.tile([128, HPC * NQT * 128], F16, tag="vb")
            kn_sb = qkv.tile([128, 2 * NQT * 128], F32, tag="kn")
            ot_sb = [otp.tile([128, S], F32R, tag=f"ot{i}", name=f"ot{i}")
                     for i in range(2)]

            nc.gpsimd.memset(vb_sb[:], 1.0)

            # ---- stage 1+2: x load/transpose + QKV projection ----
            with tc.tile_pool(name="xt", bufs=1) as xtp, \
                 tc.tile_pool(name="xload", bufs=3) as xlp, \
                 tc.tile_pool(name="tp_ps", bufs=2, space="PSUM") as tpps, \
                 tc.tile_pool(name="qk_ps", bufs=4, space="PSUM") as qkps, \
                 tc.tile_pool(name="vp_ps", bufs=2, space="PSUM") as vpps:

                xt_sb = xtp.tile([128, NDC * S], F32R, tag="xt")
                xt3 = xt_sb[:].rearrange("p (dc s) -> p dc s", dc=NDC)

                # prefetch first x tiles so PE transposes start immediately
                x_pre = {}
                for st in range(3):
                    x_t = xlp.tile([128, D], F32R, tag="x", name=f"x{st}")
                    nc.sync.dma_start(out=x_t[:],
                                      in_=xb[st * 128:(st + 1) * 128, :])
                    x_pre[st] = x_t

                # weights next in emission (three 1MB transfers so later x
                # tiles interleave on the DMA engines)
                wqkv_sb = cpool.tile([128, 3 * NDC * FPC], F32R, tag="wqkv")
                NW = NDC * FPC
                for wi3 in range(3):
                    nc.scalar.dma_start(
                        out=wqkv_sb[:, wi3 * NW:(wi3 + 1) * NW],
                        in_=wqkv[:, wi3 * NW:(wi3 + 1) * NW])
                wq_sb = wqkv_sb[:, 0:NDC * FPC]
                wk_sb = wqkv_sb[:, NDC * FPC:2 * NDC * FPC]
                wv_sb = wqkv_sb[:, 2 * NDC * FPC:3 * NDC * FPC]
                bqk_sb = cpool.tile([128, 4], F32, tag="bqk")
                nc.scalar.dma_start(out=bqk_sb[:], in_=bqk[:])
                bv_sb = cpool.tile([1, FPC], F32R, tag="bv")
                nc.scalar.dma_start(out=bv_sb[:], in_=bv[:])
                madd_sb = cpool.tile([128, 256], F16, tag="madd")
                nc.scalar.dma_start(out=madd_sb[:], in_=madd[:])
                idh_sb = cpool.tile([128, 128], F16, tag="idh")
                nc.scalar.dma_start(out=idh_sb[:], in_=idh[:])
                on_sb = cpool.tile([1, 128], F32R, tag="on")
                nc.scalar.dma_start(out=on_sb[:], in_=onesr[:])

                # interleave x load/transpose with QKV so PE never waits
                # for the full x transfer (PE streams are in-order)
                for sc in range(4):
                    for st in range(4 * sc, 4 * sc + 4):
                        if st in x_pre:
                            x_t = x_pre.pop(st)
                        else:
                            x_t = xlp.tile([128, D], F32R, tag="x",
                                           name=f"x{st}")
                            nc.sync.dma_start(
                                out=x_t[:],
                                in_=xb[st * 128:(st + 1) * 128, :])
                        for dg in range(2):
                            ps = tpps.tile([128, 512], F32R, tag="tp",
                                           name=f"tp{st}_{dg}")
                            for j in range(4):
                                dc = dg * 4 + j
                                nc.tensor.transpose(
                                    ps[:, j * 128:(j + 1) * 128],
                                    x_t[:, dc * 128:(dc + 1) * 128],
                                    idr_sb[:])
                            dst = xt3[:, dg * 4:(dg + 1) * 4,
                                      st * 128:st * 128 + 128]
                            srcp = ps[:].rearrange("p (j s) -> p j s", j=4)
                            if (st + dg) % 2 == 0:
                                nc.vector.tensor_copy(dst, srcp)
                            else:
                                nc.scalar.copy(dst, srcp)

                    # Q^T / K^T columns for this 512-wide S group.
                    # Q evicts on ACT (+bias), K evicts on DVE (+bias).
                    for wi, (w_sb, dstt) in enumerate(((wq_sb, qt_sb),
                                                       (wk_sb, kt_sb))):
                        for ft in range(2):
                            psq = qkps.tile([128, 512], F32, tag="qk",
                                            name=f"qkps{sc}_{wi}{ft}")
                            for dc in range(NDC):
                                lhsT = w_sb[:, dc * FPC + ft * 128:
                                            dc * FPC + ft * 128 + 128]
                                nc.tensor.matmul(
                                    psq[:], lhsT,
                                    xt_sb[:, dc * S + sc * 512:
                                          dc * S + sc * 512 + 512],
                                    start=(dc == 0), stop=(dc == NDC - 1))
                            bias_ap = bqk_sb[:, 2 * wi + ft: 2 * wi + ft + 1]
                            dslc = dstt[ft][:, sc * 512:(sc + 1) * 512]
                            if wi == 0:
                                nc.scalar.activation(dslc, psq[:], Ident,
                                                     bias=bias_ap)
                            else:
                                nc.vector.tensor_scalar_add(dslc, psq[:],
                                                            bias_ap)

                    # K natural for `present`: 4 batched PE transposes
                    # per ft into one PSUM bank, single ACT evict
                    for ft in range(2):
                        psn = tpps.tile([128, 512], F32R, tag="tp",
                                        name=f"kn{sc}_{ft}")
                        for j in range(4):
                            st = 4 * sc + j
                            nc.tensor.transpose(
                                psn[:, j * 128:(j + 1) * 128],
                                kt_sb[ft][:, st * 128:(st + 1) * 128],
                                idr_sb[:])
                        nc.scalar.copy(
                            kn_sb[:, ft * S + sc * 512:
                                  ft * S + (sc + 1) * 512],
                            psn[:].bitcast(F32))

                    # V natural + fused bias for these 4 S-tiles
                    for st in range(4 * sc, 4 * sc + 4):
                        vp = vpps.tile([128, FPC], F32, tag="vp",
                                       name=f"vp{st}")
                        for dc in range(NDC):
                            nc.tensor.matmul(
                                vp[:],
                                xt_sb[:, dc * S + st * 128:
                                      dc * S + st * 128 + 128],
                                wv_sb[:, dc * FPC:(dc + 1) * FPC],
                                start=(dc == 0), stop=False)
                        nc.tensor.matmul(vp[:], on_sb[:], bv_sb[:],
                                         start=False, stop=True)
                        nc.scalar.copy(v_sb[:, st * FPC:(st + 1) * FPC],
                                       vp[:])
                        srcv = vp[:].rearrange("p (hl c) -> p hl c", hl=HPC)
                        dst3 = vb_sb[:].rearrange(
                            "p (hl t) -> p hl t", hl=HPC)[
                            :, :, st * 128: st * 128 + HD]
                        if st % 2 == 0:
                            nc.vector.tensor_copy(dst3, srcv)
                        else:
                            nc.scalar.copy(dst3, srcv)

            # pools that reuse the space freed by xt: wp, K-natural
            # collector, attention transients, out staging
            with tc.tile_pool(name="late", bufs=1) as late, \
                 tc.tile_pool(name="pt", bufs=4) as ptp, \
                 tc.tile_pool(name="rs", bufs=4) as rsp, \
                 tc.tile_pool(name="osb", bufs=2) as osbp:

                wp_sb = late.tile([128, 2 * D], F32R, tag="wp")
                nc.scalar.dma_start(out=wp_sb[:], in_=wp[:])

                # ---- stage 3: present outputs (k/v computed above) ----
                kn4 = kn_sb[:].rearrange(
                    "p (ft st h d) -> p ft st h d", ft=2, st=NQT, h=2)
                for ft in range(2):
                    for h2 in range(2):
                        nc.sync.dma_start(
                            out=pk[2 * ft + h2, :, :].rearrange(
                                "(st p) d -> p st d", p=128),
                            in_=kn4[:, ft, :, h2, :])
                v4 = v_sb[:].rearrange(
                    "p (st hl d) -> p st hl d", st=NQT, hl=HPC)
                for hl in range(HPC):
                    nc.sync.dma_start(
                        out=pv[hl, :, :].rearrange(
                            "(st p) d -> p st d", p=128),
                        in_=v4[:, :, hl, :])

                # ---- stage 4: attention per head (software-pipelined).
                # The last head also interleaves the out-projection so PE
                # fills exp-latency stalls with useful work. ----
                def attention_head(hl, stps, ops, norm_lag, per_chunk_hook):
                    ft, po = hl // 2, (hl % 2) * 64
                    kth = kt_sb[ft]
                    qth = qt_sb[ft]
                    oth = ot_sb[ft]
                    vbh = vb_sb[:, hl * NQT * 128:(hl + 1) * NQT * 128]

                    pts = [None] * NQT
                    osums = [None] * NQT

                    def qk_exp_mask(c):
                        qw = min(384, S - c * 128)
                        sp = stps.tile([128, 384], F32, tag="sp",
                                       name=f"sp{hl}_{c}")
                        nc.tensor.matmul(
                            sp[:, :qw],
                            kth[po:po + 64, c * 128:(c + 1) * 128],
                            qth[po:po + 64, c * 128:c * 128 + qw],
                            start=True, stop=False)
                        # additive -inf masks on the two triangle thirds
                        # (fp16 rank-128 matmuls, middle third untouched)
                        nc.tensor.matmul(
                            sp[:, 0:128], idh_sb[:], madd_sb[:, 0:128],
                            start=False, stop=(qw <= 256))
                        if qw > 256:
                            nc.tensor.matmul(
                                sp[:, 256:qw], idh_sb[:],
                                madd_sb[:, 128:128 + (qw - 256)],
                                start=False, stop=True)
                        pt = ptp.tile([128, 384], F16, tag="pt",
                                      name=f"pt{hl}_{c}")
                        nc.scalar.activation(pt[:, :qw], sp[:, :qw], Exp)
                        pts[c] = pt

                    def osum(c):
                        if osums[c] is None:
                            osums[c] = ops.tile([128, 128], F32, tag="o",
                                                name=f"o{hl}_{c}")
                        return osums[c]

                    def normalize(c):
                        o_cur = osums[c]
                        rec = rsp.tile([64, 128], F32, tag="rec",
                                       name=f"rec{hl}_{c}")
                        nc.vector.reciprocal(rec[:], o_cur[64:128, :])
                        nc.vector.tensor_mul(
                            oth[po:po + 64, c * 128:(c + 1) * 128],
                            o_cur[0:64, :], rec[:])
                        osums[c] = None

                    qk_exp_mask(0)
                    qk_exp_mask(1)
                    for c in range(NQT):
                        if c + 2 < NQT:
                            qk_exp_mask(c + 2)
                        pt = pts[c]
                        qw = min(384, S - c * 128)
                        lhsT = vbh[:, c * 128:(c + 1) * 128]
                        nc.tensor.matmul(osum(c)[:], lhsT, pt[:, 0:128],
                                         start=(c == 0), stop=True)
                        if c + 1 < NQT:
                            nc.tensor.matmul(osum(c + 1)[:], lhsT,
                                             pt[:, 128:256],
                                             start=(c == 0), stop=False)
                        if c + 2 < NQT and qw > 256:
                            nc.tensor.matmul(osum(c + 2)[:], lhsT,
                                             pt[:, 256:384],
                                             start=True, stop=False)
                        # normalization lags the AV matmuls so the DVE
                        # stream never round-trips against PE
                        if c >= norm_lag:
                            normalize(c - norm_lag)
                            if per_chunk_hook is not None:
                                per_chunk_hook(c - norm_lag)
                        pts[c] = None
                    for c in range(NQT - norm_lag, NQT):
                        normalize(c)
                        if per_chunk_hook is not None:
                            per_chunk_hook(c)

                osb_state = {}

                def outproj_tile(st, opps):
                    # called once per q-tile st (in order) after all heads
                    # normalized it
                    st2, i = st // 2, st % 2
                    if i == 0:
                        osb_state["t"] = osbp.tile([128, 2 * D], F32,
                                                   tag="osb",
                                                   name=f"osb{st2}")
                    o_t = osb_state["t"]
                    for half in range(2):
                        op = opps.tile([128, 512], F32, tag="op",
                                       name=f"op{st}_{half}")
                        nc.tensor.matmul(
                            op[:], ot_sb[0][:, st * 128:(st + 1) * 128],
                            wp_sb[:, half * 512: half * 512 + 512],
                            start=True, stop=False)
                        nc.tensor.matmul(
                            op[:], ot_sb[1][:, st * 128:(st + 1) * 128],
                            wp_sb[:, D + half * 512: D + half * 512 + 512],
                            start=False, stop=True)
                        dsl = o_t[:, i * D + half * 512:
                                  i * D + (half + 1) * 512]
                        if half == 0:
                            nc.scalar.copy(dsl, op[:])
                        else:
                            nc.vector.tensor_copy(dsl, op[:])
                    if i == 1:
                        nc.sync.dma_start(
                            out=outp[st2 * 256:(st2 + 1) * 256, :].rearrange(
                                "(j p) d -> p j d", p=128),
                            in_=o_t[:].rearrange("p (j d) -> p j d", j=2))

                with tc.tile_pool(name="st_ps", bufs=3, space="PSUM") as stps, \
                     tc.tile_pool(name="o_ps", bufs=5, space="PSUM") as ops:
                    for hl in range(HPC - 1):
                        attention_head(hl, stps, ops, 2, None)

                with tc.tile_pool(name="st_ps2", bufs=2, space="PSUM") as stps, \
                     tc.tile_pool(name="o_ps2", bufs=4, space="PSUM") as ops, \
                     tc.tile_pool(name="op_ps", bufs=2, space="PSUM") as opps:
                    attention_head(HPC - 1, stps, ops, 1,
                                   lambda st: outproj_tile(st, opps))

    nc.compile()
    return nc


def _prep_in_maps(x, w_attn, b_attn, w_proj):
    """Per-core input dicts (host-side sharding + layout prep)."""
    x = np.ascontiguousarray(np.asarray(x, dtype=np.float32))
    w_attn = np.asarray(w_attn, dtype=np.float32)
    b_attn = np.asarray(b_attn, dtype=np.float32)
    w_proj = np.asarray(w_proj, dtype=np.float32)

    ident = np.eye(128, dtype=np.float32)
    ql = np.arange(128)[None, :]
    kl = np.arange(128)[:, None]
    neg = np.float16(-65504.0)
    madd = np.concatenate(
        [np.where(ql >= kl, np.float16(0), neg),
         np.where(ql < kl, np.float16(0), neg)], axis=1).astype(np.float16)
    idh = np.eye(128, dtype=np.float16)
    onesr = np.ones((1, 128), dtype=np.float32)

    def chunk_w(w_cols):  # [D, FPC] -> [128, NDC*FPC]
        return w_cols.reshape(NDC, 128, FPC).transpose(1, 0, 2).reshape(
            128, NDC * FPC)

    in_maps = []
    for core in range(NCORES):
        b, hg = core // 4, core % 4
        cols = slice(hg * FPC, (hg + 1) * FPC)
        kcols = slice(D + hg * FPC, D + (hg + 1) * FPC)
        vcols = slice(2 * D + hg * FPC, 2 * D + (hg + 1) * FPC)
        rows = slice(hg * FPC, (hg + 1) * FPC)
        wqkv = np.concatenate(
            [chunk_w(w_attn[:, cols] * np.float32(SCALE)),
             chunk_w(w_attn[:, kcols]),
             chunk_w(w_attn[:, vcols])], axis=1)
        bqk = np.stack(
            [(b_attn[cols] * np.float32(SCALE)).reshape(2, 128)[0],
             (b_attn[cols] * np.float32(SCALE)).reshape(2, 128)[1],
             b_attn[kcols].reshape(2, 128)[0],
             b_attn[kcols].reshape(2, 128)[1]], axis=1)
        in_maps.append({
            "xb": x[b],
            "wqkv": np.ascontiguousarray(wqkv),
            "wp": np.ascontiguousarray(
                w_proj[rows, :].reshape(2, 128, D).transpose(1, 0, 2).reshape(
                    128, 2 * D)),
            "bqk": np.ascontiguousarray(bqk),
            "bv": b_attn[vcols].reshape(1, FPC).copy(),
            "ident_r": ident,
            "madd": madd,
            "idh": idh,
            "onesr": onesr,
        })
    return in_maps


def kernel(x, w_attn, b_attn, w_proj, b_proj):
    if "nc" not in _CACHE:
        _CACHE["nc"] = _build_program()
    nc = _CACHE["nc"]

    in_maps = _prep_in_maps(x, w_attn, b_attn, w_proj)
    res = run_bass_kernel_spmd(nc, in_maps, core_ids=list(range(NCORES)))

    b_proj = np.asarray(b_proj, dtype=np.float32)
    out = np.zeros((B, S, D), dtype=np.float32)
    present = np.zeros((B, 2, N_HEAD, S, HD), dtype=np.float32)
    for core in range(NCORES):
        b, hg = core // 4, core % 4
        r = res.results[core]
        out[b] += r["outp"]
        present[b, 0, hg * HPC:(hg + 1) * HPC] = r["pk"]
        present[b, 1, hg * HPC:(hg + 1) * HPC] = r["pv"]
    out += b_proj
    return out, present
